# revision 10
# baseline (speedup 1.0000x reference)
"""Multi-head cross-attention on 8 Trainium2 NeuronCores.

Problem shapes (hardcoded): B=4, Ld=1024, Le=2048, d_model=1024, 8 heads x 128.
Sharding: core c handles batch b=c//2 and head-group g=c%2 (4 heads each).
Each core computes q/k/v projections for its heads, attention, and a partial
output projection over its heads' value dims; the host sums the two partial
outputs per batch and adds b_o.

Inputs (x, enc, all weights) are converted to bf16 on the host: the PE runs
bf16 at the same rate as fp32r while DMA traffic halves.

Schedule: one software-pipelined stream.  Projection bank-groups (K/Q/V and
later the output projection) are queued and drained a few per attention
chunk-pair, so the PE never waits for the Act engine's exp stream and the
whole kernel is PE-bound.  DMA order is chosen so the earliest-arriving
tensors (wk, enc, wv) feed the first projection groups; V-projection runs
first among the interleaved work since attention consumes vch chunks in
order.

Softmax denominators use tall-skinny matmuls: pT is the *stationary* operand
and a ones column moves, so each [128,1] per-query partial sum costs ~1 PE
row instead of the 512 rows a [1,512] ones-stationary layout costs.  The
per-(q2,h) denominator column [128,4] is reciprocal'd, PE-transposed into a
[1,512] row, broadcast across partitions on gpsimd, and multiplied into the
PV accumulator on the DVE.

Exps are issued per chunk-pair over a [128,1024] two-bank PSUM span, cutting
the Act engine's fixed access overhead per chunk in half.

PSUM banks: 0-1 score pairs, 2-3 PV accumulators (stage parity), 4-5
denominator columns / transpose rows (stage parity), 6-7 projection and
output-projection accumulators (rotating).
"""

import math
import sys

import numpy as np

for _p in ("/opt/trn_rl_repo", "/root/.axon_site/_ro/trn_rl_repo"):
    if _p not in sys.path:
        sys.path.append(_p)

B = 4
LQ = 1024
LK = 2048
D = 1024
H = 8
DH = 128
P = 128
HPC = 4          # heads per core
OQ = HPC * DH    # 512 projected dims per core
NQ = 512         # matmul moving free dim
KC = D // P      # 8 contraction chunks for projections
LKC = LK // P    # 16 key chunks
N_CORES = 8

_BUILT = {}


def _build(masked):
    import concourse.bass as bass  # noqa: F401
    import concourse.tile as tile
    import concourse.mybir as mybir
    from concourse import bacc

    f32 = mybir.dt.float32
    f32r = mybir.dt.float32r
    bf16 = mybir.dt.bfloat16
    Exp = mybir.ActivationFunctionType.Exp
    Copy = mybir.ActivationFunctionType.Copy

    nc = bacc.Bacc("TRN2", target_bir_lowering=False, debug=False,
                   num_devices=N_CORES)

    xT = nc.dram_tensor("xT", [D, LQ], bf16, kind="ExternalInput").ap()
    encT = nc.dram_tensor("encT", [D, LK], bf16, kind="ExternalInput").ap()
    wkT = nc.dram_tensor("wkT", [D, OQ], bf16, kind="ExternalInput").ap()
    # w_v and w_q interleaved per d-chunk.
    wvqT = nc.dram_tensor("wvqT", [D, 2 * OQ], bf16, kind="ExternalInput").ap()
    woT = nc.dram_tensor("woT", [OQ, D], bf16, kind="ExternalInput").ap()
    bq_d = nc.dram_tensor("bq", [P, HPC], f32, kind="ExternalInput").ap()
    bk_d = nc.dram_tensor("bk", [P, HPC], f32, kind="ExternalInput").ap()
    bvb_d = nc.dram_tensor("bvb", [P, OQ], f32, kind="ExternalInput").ap()
    ones_d = nc.dram_tensor("ones", [P, 1], bf16, kind="ExternalInput").ap()
    ident_d = nc.dram_tensor("identT", [P, P], f32, kind="ExternalInput").ap()
    if masked:
        maskT = nc.dram_tensor("maskT", [LK, LQ], f32, kind="ExternalInput").ap()
    out_d = nc.dram_tensor("out", [LQ, D], bf16, kind="ExternalOutput").ap()

    with tile.TileContext(nc) as tc:
        with tc.tile_pool(name="persist", bufs=1) as persist:
            qT = [persist.tile([P, LQ], f32r, name=f"qT{h}") for h in range(HPC)]
            kT = [persist.tile([P, LK], f32r, name=f"kT{h}") for h in range(HPC)]
            vch = [persist.tile([P, OQ], bf16, name=f"v{j}") for j in range(LKC)]
            bq_sb = persist.tile([P, HPC], f32, name="bq")
            bk_sb = persist.tile([P, HPC], f32, name="bk")
            bv_sb = persist.tile([P, OQ], f32, name="bvb")
            ones_col = persist.tile([P, 1], bf16, name="ones")
            ident = persist.tile([P, P], f32, name="ident")
            warm = persist.tile([1, HPC], f32, name="warm")
            wkc = [persist.tile([P, OQ], bf16, name=f"wk{d}") for d in range(KC)]
            wvqc = [persist.tile([P, 2 * OQ], bf16, name=f"wvq{d}")
                    for d in range(KC)]
            woch = [persist.tile([P, D], bf16, name=f"wo{h}")
                    for h in range(HPC)]
            valsT = [persist.tile([P, LQ], bf16, name=f"valsT{h}")
                     for h in range(HPC)]

            with (
                tc.tile_pool(name="acc", bufs=1, space="PSUM") as acc,
                tc.tile_pool(name="encp", bufs=1) as encp,
                tc.tile_pool(name="xh", bufs=1) as xhp,
                tc.tile_pool(name="pTp", bufs=3) as pTp,
                tc.tile_pool(name="smallp", bufs=2) as smallp,
                tc.tile_pool(name="maskp", bufs=16 if masked else 1) as maskp,
                tc.tile_pool(name="osb", bufs=4) as osb,
            ):
                # PSUM: scp = banks 0-1, pv = 2-3, db = 4-5, pj = 6-7.
                scp = acc.tile([P, 2 * NQ], f32, name="scp")
                pv = [acc.tile([P, NQ], f32, name=f"pv{t}") for t in range(2)]
                db = [acc.tile([P, NQ], f32, name=f"db{t}") for t in range(2)]
                pj = [acc.tile([P, NQ], f32, name=f"pj{t}") for t in range(2)]
                # enc in lk-quadrants of 512: e[lk4][d] is [128, 512].
                e = [[encp.tile([P, NQ], bf16, name=f"e{q}_{d}")
                      for d in range(KC)] for q in range(4)]
                # x in q2-halves of 512.
                xf = [[xhp.tile([P, NQ], bf16, name=f"x{q}_{d}")
                       for d in range(KC)] for q in range(2)]

                # ---- DMA issue order == service order. V path first (wk,
                # e_q0, wv), so the PE has continuous projection work while
                # wq/x stream in.
                for d in range(KC):
                    nc.sync.dma_start(wkc[d][:], wkT[d * P:(d + 1) * P, :])
                for t, src in ((bk_sb, bk_d), (bv_sb, bvb_d), (bq_sb, bq_d),
                               (ones_col, ones_d), (ident, ident_d)):
                    nc.sync.dma_start(t[:], src[:])
                for d in range(KC):
                    nc.sync.dma_start(e[0][d][:], encT[d * P:(d + 1) * P, :NQ])
                    if d == 0:
                        # Warm-up: absorb the PE p-state ramp on 1-column
                        # matmuls while DMAs stream.
                        for _ in range(4):
                            nc.tensor.matmul(
                                pj[1][:1, :1],
                                wkc[0][:, :1], wkc[0][:, :1],
                                start=True, stop=True)
                # Preload the Exp table while the PE projects.
                nc.scalar.activation(warm[:], bq_sb[:1, :], Exp)
                for d in range(KC):
                    nc.sync.dma_start(wvqc[d][:, :OQ],
                                      wvqT[d * P:(d + 1) * P, :OQ])
                for d in range(KC):
                    nc.sync.dma_start(e[1][d][:],
                                      encT[d * P:(d + 1) * P, NQ:2 * NQ])
                for d in range(KC):
                    nc.sync.dma_start(wvqc[d][:, OQ:],
                                      wvqT[d * P:(d + 1) * P, OQ:])
                for d in range(KC):
                    nc.sync.dma_start(xf[0][d][:], xT[d * P:(d + 1) * P, :NQ])
                for d in range(KC):
                    nc.sync.dma_start(e[2][d][:],
                                      encT[d * P:(d + 1) * P, 2 * NQ:3 * NQ])
                for d in range(KC):
                    nc.sync.dma_start(e[3][d][:],
                                      encT[d * P:(d + 1) * P, 3 * NQ:])
                for d in range(KC):
                    nc.sync.dma_start(xf[1][d][:], xT[d * P:(d + 1) * P, NQ:])
                for c in range(HPC):
                    nc.sync.dma_start(woch[c][:], woT[c * P:(c + 1) * P, :])

                # ---- projection bank-group emitters (banks 6-7 rotating)
                nbg = [0]

                def next_pj():
                    bank = pj[nbg[0] % 2]
                    nbg[0] += 1
                    return bank

                def kproj_group(h, lk):
                    bank = next_pj()
                    for d in range(KC):
                        nc.tensor.matmul(
                            bank[:],
                            wkc[d][:, h * DH:(h + 1) * DH],
                            e[lk][d][:],
                            start=(d == 0), stop=(d == KC - 1))
                    nc.vector.tensor_scalar_add(
                        kT[h][:, lk * NQ:(lk + 1) * NQ], bank[:],
                        bk_sb[:, h:h + 1])

                def qproj_group(h, q2):
                    bank = next_pj()
                    for d in range(KC):
                        nc.tensor.matmul(
                            bank[:],
                            wvqc[d][:, OQ + h * DH:OQ + (h + 1) * DH],
                            xf[q2][d][:],
                            start=(d == 0), stop=(d == KC - 1))
                    nc.vector.tensor_scalar_add(
                        qT[h][:, q2 * NQ:(q2 + 1) * NQ], bank[:],
                        bq_sb[:, h:h + 1])

                def vproj_group(j):
                    bank = next_pj()
                    for d in range(KC):
                        nc.tensor.matmul(
                            bank[:],
                            e[j // 4][d][:, (j % 4) * P:(j % 4 + 1) * P],
                            wvqc[d][:, :OQ],
                            start=(d == 0), stop=(d == KC - 1))
                    nc.vector.tensor_add(vch[j][:], bank[:], bv_sb[:])

                def oproj_group(lqc, o2, n):
                    bank = next_pj()
                    for h in range(HPC):
                        nc.tensor.matmul(
                            bank[:],
                            valsT[h][:, lqc * P:(lqc + 1) * P],
                            woch[h][:, o2 * NQ:(o2 + 1) * NQ],
                            start=(h == 0), stop=(h == HPC - 1))
                    ot = osb.tile([P, NQ], bf16, name="ot")
                    if n % 2 == 0:
                        nc.vector.tensor_copy(ot[:], bank[:])
                    else:
                        nc.scalar.activation(ot[:], bank[:], Copy)
                    nc.sync.dma_start(
                        out_d[lqc * P:(lqc + 1) * P,
                              o2 * NQ:(o2 + 1) * NQ], ot[:])

                # ---- attention emitters
                mask_tiles = [None] * LKC

                def attn_pair(q2, h, jp, idx):
                    """Chunks j=2jp,2jp+1: scores into the 2-bank scp span,
                    one exp over both, then PV + denominator matmuls."""
                    ps_v = pv[idx % 2]
                    dbank = db[idx % 2]
                    for t in range(2):
                        j = 2 * jp + t
                        nc.tensor.matmul(
                            scp[:, t * NQ:(t + 1) * NQ],
                            kT[h][:, j * P:(j + 1) * P],
                            qT[h][:, q2 * NQ:(q2 + 1) * NQ],
                            start=True, stop=True)
                        if masked:
                            nc.vector.tensor_add(
                                scp[:, t * NQ:(t + 1) * NQ],
                                scp[:, t * NQ:(t + 1) * NQ],
                                mask_tiles[j][:])
                    pT2 = pTp.tile([P, 2 * NQ], bf16, name="pT2")
                    nc.scalar.activation(pT2[:], scp[:], Exp)
                    for t in range(2):
                        j = 2 * jp + t
                        nc.tensor.matmul(
                            ps_v[:],
                            vch[j][:, h * DH:(h + 1) * DH],
                            pT2[:, t * NQ:(t + 1) * NQ],
                            start=(j == 0), stop=(j == LKC - 1))
                        for s in range(4):
                            nc.tensor.matmul(
                                dbank[:, s:s + 1],
                                pT2[:, t * NQ + s * P:t * NQ + (s + 1) * P],
                                ones_col[:],
                                start=(j == 0 and s == 0),
                                stop=(j == LKC - 1 and s == 3),
                                skip_group_check=True)

                def attn_norm(q2, h, idx):
                    """1/d -> [1,512] row via PE transposes -> partition
                    broadcast -> scale the PV accumulator into valsT."""
                    ps_v = pv[idx % 2]
                    dbank = db[idx % 2]
                    bankT = db[(idx + 1) % 2]
                    rsb = smallp.tile([P, 4], f32, name="rsb")
                    nc.vector.reciprocal(rsb[:], dbank[:, 0:4])
                    for s in range(4):
                        nc.tensor.transpose(
                            bankT[0:1, s * P:(s + 1) * P],
                            rsb[:, s:s + 1], ident[:])
                    rrow = smallp.tile([1, NQ], f32, name="rrow")
                    nc.vector.tensor_copy(rrow[:], bankT[0:1, :])
                    bcast = smallp.tile([P, NQ], f32, name="bcast")
                    nc.gpsimd.partition_broadcast(bcast[:], rrow[:])
                    nc.vector.tensor_mul(
                        valsT[h][:, q2 * NQ:(q2 + 1) * NQ],
                        ps_v[:], bcast[:])

                # ---- the pipelined schedule
                # Pre-stage: first K group, the full V stream, then Q(h0,0)
                # (wq/x arrive after wv in DMA order).
                kproj_group(0, 0)
                for j in range(8):
                    vproj_group(j)
                qproj_group(0, 0)

                # Per-stage projection/out-proj work queues, consumed a few
                # groups per attention pair so everything overlaps.
                stage_work = {
                    0: [lambda: kproj_group(0, 1), lambda: vproj_group(8),
                        lambda: vproj_group(9), lambda: kproj_group(0, 2),
                        lambda: vproj_group(10), lambda: vproj_group(11),
                        lambda: kproj_group(0, 3), lambda: vproj_group(12),
                        lambda: vproj_group(13), lambda: kproj_group(1, 0),
                        lambda: vproj_group(14), lambda: vproj_group(15),
                        lambda: kproj_group(1, 1), lambda: kproj_group(1, 2),
                        lambda: kproj_group(1, 3), lambda: qproj_group(1, 0),
                        lambda: qproj_group(0, 1), lambda: qproj_group(1, 1),
                        lambda: kproj_group(2, 0), lambda: kproj_group(2, 1),
                        lambda: kproj_group(2, 2), lambda: kproj_group(2, 3),
                        lambda: qproj_group(2, 0), lambda: qproj_group(2, 1)],
                    1: [lambda: kproj_group(3, 0), lambda: kproj_group(3, 1),
                        lambda: kproj_group(3, 2), lambda: kproj_group(3, 3),
                        lambda: qproj_group(3, 0), lambda: qproj_group(3, 1)],
                    4: [(lambda lqc=lqc, o2=o2:
                         oproj_group(lqc, o2, 2 * lqc + o2))
                        for lqc in range(4) for o2 in range(2)],
                }
                per_slot = {0: 3, 1: 1, 4: 1}

                for idx in range(8):
                    q2, h = idx // 4, idx % 4
                    if masked and h == 0:
                        for j in range(LKC):
                            mt = maskp.tile([P, NQ], f32, name=f"m{j}")
                            nc.sync.dma_start(
                                mt[:],
                                maskT[j * P:(j + 1) * P,
                                      q2 * NQ:(q2 + 1) * NQ])
                            mask_tiles[j] = mt
                    work = stage_work.get(idx, [])
                    rate = per_slot.get(idx, 1)
                    wi = 0
                    for jp in range(LKC // 2):
                        if idx < 4:
                            # projections must lead the attention that
                            # consumes them
                            for _ in range(rate):
                                if wi < len(work):
                                    work[wi]()
                                    wi += 1
                            attn_pair(q2, h, jp, idx)
                        else:
                            # out-proj trails the attention (valsT of the
                            # previous stage must be normalized first)
                            attn_pair(q2, h, jp, idx)
                            for _ in range(rate):
                                if wi < len(work):
                                    work[wi]()
                                    wi += 1
                    while wi < len(work):
                        work[wi]()
                        wi += 1
                    attn_norm(q2, h, idx)

                # Tail: second-half output projection.
                for lqc in range(4, 8):
                    for o2 in range(2):
                        oproj_group(lqc, o2, 2 * lqc + o2)

    nc.compile()
    return nc


def _get_built(masked):
    if masked not in _BUILT:
        _BUILT[masked] = _build(masked)
    return _BUILT[masked]


def _shard_inputs(inputs, masked):
    import ml_dtypes

    bf16 = ml_dtypes.bfloat16

    x = np.asarray(inputs["mhca_input"], np.float32)
    enc = np.asarray(inputs["encoder_output"], np.float32)
    mask = np.asarray(inputs["cross_mask"], np.float32)
    W_kv = np.asarray(inputs["W_kv"], np.float32)
    b_kv = np.asarray(inputs["b_kv"], np.float32)
    W_q = np.asarray(inputs["W_q"], np.float32)
    b_q = np.asarray(inputs["b_q"], np.float32)
    W_o = np.asarray(inputs["W_o"], np.float32)

    scale = 1.0 / math.sqrt(DH)
    in_maps = []
    for c in range(N_CORES):
        b = c // 2
        g = c % 2
        heads = list(range(g * HPC, (g + 1) * HPC))
        sl = slice(g * OQ, (g + 1) * OQ)
        k_rows = np.concatenate(
            [W_kv[h * 2 * DH:h * 2 * DH + DH] for h in heads], 0)
        v_rows = np.concatenate(
            [W_kv[h * 2 * DH + DH:(h + 1) * 2 * DH] for h in heads], 0)
        bv_rows = np.concatenate(
            [b_kv[h * 2 * DH + DH:(h + 1) * 2 * DH] for h in heads], 0)
        wvq = np.concatenate([v_rows.T, (W_q[sl] * scale).T], axis=1)
        m = {
            "xT": np.ascontiguousarray(x[b].T).astype(bf16),
            "encT": np.ascontiguousarray(enc[b].T).astype(bf16),
            "wkT": np.ascontiguousarray(k_rows.T).astype(bf16),
            "wvqT": np.ascontiguousarray(wvq).astype(bf16),
            "woT": np.ascontiguousarray(W_o[:, sl].T).astype(bf16),
            "bq": np.ascontiguousarray((b_q[sl] * scale).reshape(HPC, DH).T),
            "bk": np.ascontiguousarray(
                np.stack([b_kv[h * 2 * DH:h * 2 * DH + DH] for h in heads], 1)),
            "bvb": np.ascontiguousarray(
                np.tile(bv_rows[None, :], (P, 1)).astype(np.float32)),
            "ones": np.ones((P, 1), bf16),
            "identT": np.eye(P, dtype=np.float32),
        }
        if masked:
            m["maskT"] = np.ascontiguousarray(mask[b].T)
        in_maps.append(m)
    return in_maps


def kernel(mhca_input, encoder_output, cross_mask, W_kv, b_kv, W_q, b_q, W_o,
           b_o):
    from concourse.bass_utils import run_bass_kernel_spmd

    inputs = {
        "mhca_input": mhca_input, "encoder_output": encoder_output,
        "cross_mask": cross_mask, "W_kv": W_kv, "b_kv": b_kv, "W_q": W_q,
        "b_q": b_q, "W_o": W_o,
    }
    b_o = np.asarray(b_o, np.float32)
    masked = bool(np.any(np.asarray(cross_mask)))
    nc = _get_built(masked)
    in_maps = _shard_inputs(inputs, masked)

    res = run_bass_kernel_spmd(nc, in_maps, core_ids=list(range(N_CORES)))
    outs = [np.asarray(res.results[c]["out"], np.float32)
            for c in range(N_CORES)]
    full = np.stack([outs[2 * b] + outs[2 * b + 1] for b in range(B)], 0)
    return (full + b_o[None, None, :]).astype(np.float32)


# revision 12
# speedup vs baseline: 1.1964x; 1.1964x over previous
"""Multi-head cross-attention on 8 Trainium2 NeuronCores.

Problem shapes (hardcoded): B=4, Ld=1024, Le=2048, d_model=1024, 8 heads x 128.
Sharding: core c handles batch b=c//2 and head-group g=c%2 (4 heads each).
Each core computes q/k/v projections for its heads, attention, and a partial
output projection over its heads' value dims; the host sums the two partial
outputs per batch and adds b_o.

Inputs (x, enc, all weights) are converted to bf16 on the host: the PE runs
bf16 at the same rate as fp32r while DMA traffic halves.

Schedule: one software-pipelined stream.  Projection bank-groups (K/Q/V and
later the output projection) are queued and drained a few per attention
chunk-pair, so the PE never waits for the Act engine's exp stream and the
whole kernel is PE-bound.  DMA order is chosen so the earliest-arriving
tensors (wk, enc, wv) feed the first projection groups; V-projection runs
first among the interleaved work since attention consumes vch chunks in
order.

Softmax denominators use tall-skinny matmuls: pT is the *stationary* operand
and a ones column moves, so each [128,1] per-query partial sum costs ~1 PE
row instead of the 512 rows a [1,512] ones-stationary layout costs.  The
per-(q2,h) denominator column [128,4] is reciprocal'd, PE-transposed into a
[1,512] row, broadcast across partitions on gpsimd, and multiplied into the
PV accumulator on the DVE.

Exps are issued per chunk-pair over a [128,1024] two-bank PSUM span, cutting
the Act engine's fixed access overhead per chunk in half.

PSUM banks: 0-1 score pairs, 2-3 PV accumulators (stage parity), 4-5
denominator columns / transpose rows (stage parity), 6-7 projection and
output-projection accumulators (rotating).
"""

import math
import sys

import numpy as np

for _p in ("/opt/trn_rl_repo", "/root/.axon_site/_ro/trn_rl_repo"):
    if _p not in sys.path:
        sys.path.append(_p)

B = 4
LQ = 1024
LK = 2048
D = 1024
H = 8
DH = 128
P = 128
HPC = 4          # heads per core
OQ = HPC * DH    # 512 projected dims per core
NQ = 512         # matmul moving free dim
KC = D // P      # 8 contraction chunks for projections
LKC = LK // P    # 16 key chunks
N_CORES = 8

_BUILT = {}


def _build(masked):
    import concourse.bass as bass  # noqa: F401
    import concourse.tile as tile
    import concourse.mybir as mybir
    from concourse import bacc

    f32 = mybir.dt.float32
    f32r = mybir.dt.float32r
    bf16 = mybir.dt.bfloat16
    Exp = mybir.ActivationFunctionType.Exp
    Copy = mybir.ActivationFunctionType.Copy

    nc = bacc.Bacc("TRN2", target_bir_lowering=False, debug=False,
                   num_devices=N_CORES)

    xT = nc.dram_tensor("xT", [D, LQ], bf16, kind="ExternalInput").ap()
    encT = nc.dram_tensor("encT", [D, LK], bf16, kind="ExternalInput").ap()
    wkT = nc.dram_tensor("wkT", [D, OQ], bf16, kind="ExternalInput").ap()
    # w_v and w_q interleaved per d-chunk.
    wvqT = nc.dram_tensor("wvqT", [D, 2 * OQ], bf16, kind="ExternalInput").ap()
    woT = nc.dram_tensor("woT", [OQ, D], bf16, kind="ExternalInput").ap()
    bq_d = nc.dram_tensor("bq", [P, HPC], f32, kind="ExternalInput").ap()
    bk_d = nc.dram_tensor("bk", [P, HPC], f32, kind="ExternalInput").ap()
    bvb_d = nc.dram_tensor("bvb", [P, OQ], f32, kind="ExternalInput").ap()
    ones_d = nc.dram_tensor("ones", [P, 1], bf16, kind="ExternalInput").ap()
    ident_d = nc.dram_tensor("identT", [P, P], f32, kind="ExternalInput").ap()
    if masked:
        maskT = nc.dram_tensor("maskT", [LK, LQ], f32, kind="ExternalInput").ap()
    out_d = nc.dram_tensor("out", [LQ, D], bf16, kind="ExternalOutput").ap()

    with tile.TileContext(nc) as tc:
        with tc.tile_pool(name="persist", bufs=1) as persist:
            qT = [persist.tile([P, LQ], f32r, name=f"qT{h}") for h in range(HPC)]
            kT = [persist.tile([P, LK], f32r, name=f"kT{h}") for h in range(HPC)]
            vch = [persist.tile([P, OQ], bf16, name=f"v{j}") for j in range(LKC)]
            bq_sb = persist.tile([P, HPC], f32, name="bq")
            bk_sb = persist.tile([P, HPC], f32, name="bk")
            bv_sb = persist.tile([P, OQ], f32, name="bvb")
            ones_col = persist.tile([P, 1], bf16, name="ones")
            ident = persist.tile([P, P], f32, name="ident")
            warm = persist.tile([1, HPC], f32, name="warm")
            wkc = [persist.tile([P, OQ], bf16, name=f"wk{d}") for d in range(KC)]
            wvqc = [persist.tile([P, 2 * OQ], bf16, name=f"wvq{d}")
                    for d in range(KC)]
            woch = [persist.tile([P, D], bf16, name=f"wo{h}")
                    for h in range(HPC)]
            valsT = [persist.tile([P, LQ], bf16, name=f"valsT{h}")
                     for h in range(HPC)]

            with (
                tc.tile_pool(name="acc", bufs=1, space="PSUM") as acc,
                tc.tile_pool(name="encp", bufs=1) as encp,
                tc.tile_pool(name="xh", bufs=1) as xhp,
                tc.tile_pool(name="pTp", bufs=3) as pTp,
                tc.tile_pool(name="smallp", bufs=2) as smallp,
                tc.tile_pool(name="maskp", bufs=16 if masked else 1) as maskp,
                tc.tile_pool(name="osb", bufs=4) as osb,
            ):
                # PSUM: two score pair-spans (banks 0-3) so scores for pair
                # jp+1 never wait on exp(jp); single PV accumulator (bank 4,
                # drained by the normalize-mul); denominator columns bank 5;
                # projection/out-proj accumulators banks 6-7.
                sp = [acc.tile([P, 2 * NQ], f32, name=f"sp{t}")
                      for t in range(2)]
                pvb = acc.tile([P, NQ], f32, name="pvb")
                dbk = acc.tile([P, NQ], f32, name="dbk")
                pj = [acc.tile([P, NQ], f32, name=f"pj{t}") for t in range(2)]
                # enc in lk-quadrants of 512: e[lk4][d] is [128, 512].
                e = [[encp.tile([P, NQ], bf16, name=f"e{q}_{d}")
                      for d in range(KC)] for q in range(4)]
                # x in q2-halves of 512.
                xf = [[xhp.tile([P, NQ], bf16, name=f"x{q}_{d}")
                       for d in range(KC)] for q in range(2)]

                # ---- DMA issue order == service order. V path first (wk,
                # e_q0, wv), so the PE has continuous projection work while
                # wq/x stream in.
                for d in range(KC):
                    nc.sync.dma_start(wkc[d][:], wkT[d * P:(d + 1) * P, :])
                for t, src in ((bk_sb, bk_d), (bv_sb, bvb_d), (bq_sb, bq_d),
                               (ones_col, ones_d), (ident, ident_d)):
                    nc.sync.dma_start(t[:], src[:])
                for d in range(KC):
                    nc.sync.dma_start(e[0][d][:], encT[d * P:(d + 1) * P, :NQ])
                    if d == 0:
                        # Warm-up: absorb the PE p-state ramp on 1-column
                        # matmuls while DMAs stream.
                        for _ in range(4):
                            nc.tensor.matmul(
                                pj[1][:1, :1],
                                wkc[0][:, :1], wkc[0][:, :1],
                                start=True, stop=True)
                # Preload the Exp table while the PE projects.
                nc.scalar.activation(warm[:], bq_sb[:1, :], Exp)
                for d in range(KC):
                    nc.sync.dma_start(wvqc[d][:, :OQ],
                                      wvqT[d * P:(d + 1) * P, :OQ])
                for d in range(KC):
                    nc.sync.dma_start(e[1][d][:],
                                      encT[d * P:(d + 1) * P, NQ:2 * NQ])
                for d in range(KC):
                    nc.sync.dma_start(wvqc[d][:, OQ:],
                                      wvqT[d * P:(d + 1) * P, OQ:])
                for d in range(KC):
                    nc.sync.dma_start(xf[0][d][:], xT[d * P:(d + 1) * P, :NQ])
                for d in range(KC):
                    nc.sync.dma_start(e[2][d][:],
                                      encT[d * P:(d + 1) * P, 2 * NQ:3 * NQ])
                for d in range(KC):
                    nc.sync.dma_start(e[3][d][:],
                                      encT[d * P:(d + 1) * P, 3 * NQ:])
                for d in range(KC):
                    nc.sync.dma_start(xf[1][d][:], xT[d * P:(d + 1) * P, NQ:])
                for c in range(HPC):
                    nc.sync.dma_start(woch[c][:], woT[c * P:(c + 1) * P, :])

                # ---- projection bank-group emitters (banks 6-7 rotating)
                nbg = [0]

                def next_pj():
                    bank = pj[nbg[0] % 2]
                    nbg[0] += 1
                    return bank

                def kproj_group(h, lk):
                    bank = next_pj()
                    for d in range(KC):
                        nc.tensor.matmul(
                            bank[:],
                            wkc[d][:, h * DH:(h + 1) * DH],
                            e[lk][d][:],
                            start=(d == 0), stop=(d == KC - 1))
                    nc.vector.tensor_scalar_add(
                        kT[h][:, lk * NQ:(lk + 1) * NQ], bank[:],
                        bk_sb[:, h:h + 1])

                def qproj_group(h, q2):
                    bank = next_pj()
                    for d in range(KC):
                        nc.tensor.matmul(
                            bank[:],
                            wvqc[d][:, OQ + h * DH:OQ + (h + 1) * DH],
                            xf[q2][d][:],
                            start=(d == 0), stop=(d == KC - 1))
                    nc.vector.tensor_scalar_add(
                        qT[h][:, q2 * NQ:(q2 + 1) * NQ], bank[:],
                        bq_sb[:, h:h + 1])

                def vproj_group(j):
                    bank = next_pj()
                    for d in range(KC):
                        nc.tensor.matmul(
                            bank[:],
                            e[j // 4][d][:, (j % 4) * P:(j % 4 + 1) * P],
                            wvqc[d][:, :OQ],
                            start=(d == 0), stop=(d == KC - 1))
                    nc.vector.tensor_add(vch[j][:], bank[:], bv_sb[:])

                def oproj_group(lqc, o2, n):
                    bank = next_pj()
                    for h in range(HPC):
                        nc.tensor.matmul(
                            bank[:],
                            valsT[h][:, lqc * P:(lqc + 1) * P],
                            woch[h][:, o2 * NQ:(o2 + 1) * NQ],
                            start=(h == 0), stop=(h == HPC - 1))
                    ot = osb.tile([P, NQ], bf16, name="ot")
                    if n % 2 == 0:
                        nc.vector.tensor_copy(ot[:], bank[:])
                    else:
                        nc.scalar.activation(ot[:], bank[:], Copy)
                    nc.sync.dma_start(
                        out_d[lqc * P:(lqc + 1) * P,
                              o2 * NQ:(o2 + 1) * NQ], ot[:])

                # ---- attention emitters
                mask_tiles = [None] * LKC

                def attn_pair(q2, h, jp, idx):
                    """Chunks j=2jp,2jp+1: scores into the pair-span jp%2,
                    one exp over both, then PV + denominator matmuls."""
                    span = sp[jp % 2]
                    for t in range(2):
                        j = 2 * jp + t
                        nc.tensor.matmul(
                            span[:, t * NQ:(t + 1) * NQ],
                            kT[h][:, j * P:(j + 1) * P],
                            qT[h][:, q2 * NQ:(q2 + 1) * NQ],
                            start=True, stop=True)
                        if masked:
                            nc.vector.tensor_add(
                                span[:, t * NQ:(t + 1) * NQ],
                                span[:, t * NQ:(t + 1) * NQ],
                                mask_tiles[j][:])
                    pT2 = pTp.tile([P, 2 * NQ], bf16, name="pT2")
                    nc.scalar.activation(pT2[:], span[:], Exp)
                    for t in range(2):
                        j = 2 * jp + t
                        nc.tensor.matmul(
                            pvb[:],
                            vch[j][:, h * DH:(h + 1) * DH],
                            pT2[:, t * NQ:(t + 1) * NQ],
                            start=(j == 0), stop=(j == LKC - 1))
                        for s in range(4):
                            nc.tensor.matmul(
                                dbk[:, s:s + 1],
                                pT2[:, t * NQ + s * P:t * NQ + (s + 1) * P],
                                ones_col[:],
                                start=(j == 0 and s == 0),
                                stop=(j == LKC - 1 and s == 3),
                                skip_group_check=True)

                def attn_norm(q2, h, idx):
                    """1/d -> [1,512] row via PE transposes (into the dead
                    half of span B) -> partition broadcast -> scale the PV
                    accumulator into valsT (this is also the PV drain)."""
                    rsb = smallp.tile([P, 4], f32, name="rsb")
                    nc.vector.reciprocal(rsb[:], dbk[:, 0:4])
                    for s in range(4):
                        nc.tensor.transpose(
                            sp[1][0:1, NQ + s * P:NQ + (s + 1) * P],
                            rsb[:, s:s + 1], ident[:])
                    rrow = smallp.tile([1, NQ], f32, name="rrow")
                    nc.vector.tensor_copy(rrow[:], sp[1][0:1, NQ:])
                    bcast = smallp.tile([P, NQ], f32, name="bcast")
                    nc.gpsimd.partition_broadcast(bcast[:], rrow[:])
                    nc.vector.tensor_mul(
                        valsT[h][:, q2 * NQ:(q2 + 1) * NQ],
                        pvb[:], bcast[:])

                # ---- the pipelined schedule
                # Pre-stage: first K group, the full V stream, then Q(h0,0)
                # (wq/x arrive after wv in DMA order).
                kproj_group(0, 0)
                for j in range(8):
                    vproj_group(j)
                qproj_group(0, 0)

                # Per-stage projection/out-proj work queues, consumed a few
                # groups per attention pair so everything overlaps.
                stage_work = {
                    0: [lambda: kproj_group(0, 1), lambda: vproj_group(8),
                        lambda: vproj_group(9), lambda: kproj_group(0, 2),
                        lambda: vproj_group(10), lambda: vproj_group(11),
                        lambda: kproj_group(0, 3), lambda: vproj_group(12),
                        lambda: vproj_group(13), lambda: kproj_group(1, 0),
                        lambda: vproj_group(14), lambda: vproj_group(15),
                        lambda: kproj_group(1, 1), lambda: kproj_group(1, 2),
                        lambda: kproj_group(1, 3), lambda: qproj_group(1, 0),
                        lambda: qproj_group(0, 1), lambda: qproj_group(1, 1),
                        lambda: kproj_group(2, 0), lambda: kproj_group(2, 1),
                        lambda: kproj_group(2, 2), lambda: kproj_group(2, 3),
                        lambda: qproj_group(2, 0), lambda: qproj_group(2, 1)],
                    1: [lambda: kproj_group(3, 0), lambda: kproj_group(3, 1),
                        lambda: kproj_group(3, 2), lambda: kproj_group(3, 3),
                        lambda: qproj_group(3, 0), lambda: qproj_group(3, 1)],
                    4: [(lambda lqc=lqc, o2=o2:
                         oproj_group(lqc, o2, 2 * lqc + o2))
                        for lqc in range(4) for o2 in range(2)],
                }
                per_slot = {0: 3, 1: 1, 4: 1}

                for idx in range(8):
                    q2, h = idx // 4, idx % 4
                    if masked and h == 0:
                        for j in range(LKC):
                            mt = maskp.tile([P, NQ], f32, name=f"m{j}")
                            nc.sync.dma_start(
                                mt[:],
                                maskT[j * P:(j + 1) * P,
                                      q2 * NQ:(q2 + 1) * NQ])
                            mask_tiles[j] = mt
                    work = stage_work.get(idx, [])
                    rate = per_slot.get(idx, 1)
                    wi = 0
                    for jp in range(LKC // 2):
                        if idx < 4:
                            # projections must lead the attention that
                            # consumes them
                            for _ in range(rate):
                                if wi < len(work):
                                    work[wi]()
                                    wi += 1
                            attn_pair(q2, h, jp, idx)
                        else:
                            # out-proj trails the attention (valsT of the
                            # previous stage must be normalized first)
                            attn_pair(q2, h, jp, idx)
                            for _ in range(rate):
                                if wi < len(work):
                                    work[wi]()
                                    wi += 1
                    while wi < len(work):
                        work[wi]()
                        wi += 1
                    attn_norm(q2, h, idx)

                # Tail: second-half output projection.
                for lqc in range(4, 8):
                    for o2 in range(2):
                        oproj_group(lqc, o2, 2 * lqc + o2)

    nc.compile()
    return nc


def _get_built(masked):
    if masked not in _BUILT:
        _BUILT[masked] = _build(masked)
    return _BUILT[masked]


def _shard_inputs(inputs, masked):
    import ml_dtypes

    bf16 = ml_dtypes.bfloat16

    x = np.asarray(inputs["mhca_input"], np.float32)
    enc = np.asarray(inputs["encoder_output"], np.float32)
    mask = np.asarray(inputs["cross_mask"], np.float32)
    W_kv = np.asarray(inputs["W_kv"], np.float32)
    b_kv = np.asarray(inputs["b_kv"], np.float32)
    W_q = np.asarray(inputs["W_q"], np.float32)
    b_q = np.asarray(inputs["b_q"], np.float32)
    W_o = np.asarray(inputs["W_o"], np.float32)

    scale = 1.0 / math.sqrt(DH)
    in_maps = []
    for c in range(N_CORES):
        b = c // 2
        g = c % 2
        heads = list(range(g * HPC, (g + 1) * HPC))
        sl = slice(g * OQ, (g + 1) * OQ)
        k_rows = np.concatenate(
            [W_kv[h * 2 * DH:h * 2 * DH + DH] for h in heads], 0)
        v_rows = np.concatenate(
            [W_kv[h * 2 * DH + DH:(h + 1) * 2 * DH] for h in heads], 0)
        bv_rows = np.concatenate(
            [b_kv[h * 2 * DH + DH:(h + 1) * 2 * DH] for h in heads], 0)
        wvq = np.concatenate([v_rows.T, (W_q[sl] * scale).T], axis=1)
        m = {
            "xT": np.ascontiguousarray(x[b].T).astype(bf16),
            "encT": np.ascontiguousarray(enc[b].T).astype(bf16),
            "wkT": np.ascontiguousarray(k_rows.T).astype(bf16),
            "wvqT": np.ascontiguousarray(wvq).astype(bf16),
            "woT": np.ascontiguousarray(W_o[:, sl].T).astype(bf16),
            "bq": np.ascontiguousarray((b_q[sl] * scale).reshape(HPC, DH).T),
            "bk": np.ascontiguousarray(
                np.stack([b_kv[h * 2 * DH:h * 2 * DH + DH] for h in heads], 1)),
            "bvb": np.ascontiguousarray(
                np.tile(bv_rows[None, :], (P, 1)).astype(np.float32)),
            "ones": np.ones((P, 1), bf16),
            "identT": np.eye(P, dtype=np.float32),
        }
        if masked:
            m["maskT"] = np.ascontiguousarray(mask[b].T)
        in_maps.append(m)
    return in_maps


def kernel(mhca_input, encoder_output, cross_mask, W_kv, b_kv, W_q, b_q, W_o,
           b_o):
    from concourse.bass_utils import run_bass_kernel_spmd

    inputs = {
        "mhca_input": mhca_input, "encoder_output": encoder_output,
        "cross_mask": cross_mask, "W_kv": W_kv, "b_kv": b_kv, "W_q": W_q,
        "b_q": b_q, "W_o": W_o,
    }
    b_o = np.asarray(b_o, np.float32)
    masked = bool(np.any(np.asarray(cross_mask)))
    nc = _get_built(masked)
    in_maps = _shard_inputs(inputs, masked)

    res = run_bass_kernel_spmd(nc, in_maps, core_ids=list(range(N_CORES)))
    outs = [np.asarray(res.results[c]["out"], np.float32)
            for c in range(N_CORES)]
    full = np.stack([outs[2 * b] + outs[2 * b + 1] for b in range(B)], 0)
    return (full + b_o[None, None, :]).astype(np.float32)


# revision 14
# speedup vs baseline: 1.2597x; 1.0529x over previous
"""Multi-head cross-attention on 8 Trainium2 NeuronCores.

Problem shapes (hardcoded): B=4, Ld=1024, Le=2048, d_model=1024, 8 heads x 128.
Sharding: core c handles batch b=c//2 and head-group g=c%2 (4 heads each).
Each core computes q/k/v projections for its heads, attention, and a partial
output projection over its heads' value dims; the host sums the two partial
outputs per batch and adds b_o.

All inputs are host-repacked into contiguous [128, X] layouts so each tensor
is a single large DMA (descriptor-generation slots are an exclusive 625ns
resource; dozens of small DMAs serialize the front of the kernel).

Schedule: one software-pipelined stream.  Projection bank-groups (K/Q/V and
later the output projection) are queued and drained a few per attention
chunk-pair, so the PE never waits for the Act engine's exp stream and the
whole kernel is PE-bound.  DMA order feeds the V path first (wk, enc, wv),
giving the PE continuous early work while wq/x stream in.

Softmax denominators use tall-skinny matmuls: pT is the *stationary* operand
and a ones column moves, so each [128,1] per-query partial sum costs ~1 PE
row instead of the 512 rows a [1,512] ones-stationary layout costs.

Exps are issued per chunk-pair over a [128,1024] two-bank PSUM span (halving
the Act engine's fixed access overhead), double-buffered across two spans so
scores never wait on the previous exp.

The per-stage normalize chain (reciprocal -> PE-transpose to a [1,512] row
-> gpsimd partition-broadcast -> multiply) is split: the PV accumulator is
drained to SBUF raw (one DVE copy) so the next stage's PV can start
immediately, and the rest of the chain is emitted after the next stage's
first pair, fully off the PE's critical path.

PSUM banks: 0-3 two score pair-spans, 4 PV accumulator, 5 denominator
columns, 6-7 projection / output-projection accumulators (rotating).
"""

import math
import sys

import numpy as np

for _p in ("/opt/trn_rl_repo", "/root/.axon_site/_ro/trn_rl_repo"):
    if _p not in sys.path:
        sys.path.append(_p)

B = 4
LQ = 1024
LK = 2048
D = 1024
H = 8
DH = 128
P = 128
HPC = 4          # heads per core
OQ = HPC * DH    # 512 projected dims per core
NQ = 512         # matmul moving free dim
KC = D // P      # 8 contraction chunks for projections
LKC = LK // P    # 16 key chunks
N_CORES = 8

_BUILT = {}


def _repack(a):
    """[KC*128, X] -> [128, KC*X] with d-chunk-major columns."""
    kc = a.shape[0] // P
    return np.ascontiguousarray(
        a.reshape(kc, P, a.shape[1]).transpose(1, 0, 2).reshape(P, -1))


def _build(masked):
    import concourse.bass as bass  # noqa: F401
    import concourse.tile as tile
    import concourse.mybir as mybir
    from concourse import bacc

    f32 = mybir.dt.float32
    bf16 = mybir.dt.bfloat16
    f32r = mybir.dt.float32r
    Exp = mybir.ActivationFunctionType.Exp
    Copy = mybir.ActivationFunctionType.Copy

    nc = bacc.Bacc("TRN2", target_bir_lowering=False, debug=False,
                   num_devices=N_CORES)

    BW = KC * NQ  # 4096: big packed width
    x_d = [nc.dram_tensor(f"x{q}", [P, BW], bf16, kind="ExternalInput").ap()
           for q in range(2)]
    e_d = [nc.dram_tensor(f"enc{q}", [P, BW], bf16, kind="ExternalInput").ap()
           for q in range(4)]
    wk_d = nc.dram_tensor("wk", [P, BW], bf16, kind="ExternalInput").ap()
    wv_d = nc.dram_tensor("wv", [P, BW], bf16, kind="ExternalInput").ap()
    wq_d = nc.dram_tensor("wq", [P, BW], bf16, kind="ExternalInput").ap()
    wo_d = nc.dram_tensor("wo", [P, BW], bf16, kind="ExternalInput").ap()
    bq_d = nc.dram_tensor("bq", [P, HPC], f32, kind="ExternalInput").ap()
    bk_d = nc.dram_tensor("bk", [P, HPC], f32, kind="ExternalInput").ap()
    bvb_d = nc.dram_tensor("bvb", [P, OQ], f32, kind="ExternalInput").ap()
    ones_d = nc.dram_tensor("ones", [P, 1], bf16, kind="ExternalInput").ap()
    ident_d = nc.dram_tensor("identT", [P, P], f32, kind="ExternalInput").ap()
    if masked:
        maskT = nc.dram_tensor("maskT", [LK, LQ], f32, kind="ExternalInput").ap()
    out_d = nc.dram_tensor("out", [LQ, D], bf16, kind="ExternalOutput").ap()

    with tile.TileContext(nc) as tc:
        with tc.tile_pool(name="persist", bufs=1) as persist:
            qT = [persist.tile([P, LQ], f32r, name=f"qT{h}") for h in range(HPC)]
            kT = [persist.tile([P, LK], f32r, name=f"kT{h}") for h in range(HPC)]
            vch = [persist.tile([P, OQ], bf16, name=f"v{j}") for j in range(LKC)]
            bq_sb = persist.tile([P, HPC], f32, name="bq")
            bk_sb = persist.tile([P, HPC], f32, name="bk")
            bv_sb = persist.tile([P, OQ], f32, name="bvb")
            ones_col = persist.tile([P, 1], bf16, name="ones")
            ident = persist.tile([P, P], f32, name="ident")
            warm = persist.tile([1, HPC], f32, name="warm")
            wkb = persist.tile([P, BW], bf16, name="wkb")
            wvb = persist.tile([P, BW], bf16, name="wvb")
            wqb = persist.tile([P, BW], bf16, name="wqb")
            wob = persist.tile([P, BW], bf16, name="wob")
            eb = [persist.tile([P, BW], bf16, name=f"eb{q}") for q in range(4)]
            xb = [persist.tile([P, BW], bf16, name=f"xb{q}") for q in range(2)]
            valsT = [persist.tile([P, LQ], bf16, name=f"valsT{h}")
                     for h in range(HPC)]

            with (
                tc.tile_pool(name="acc", bufs=1, space="PSUM") as acc,
                tc.tile_pool(name="pTp", bufs=3) as pTp,
                tc.tile_pool(name="smallp", bufs=2) as smallp,
                tc.tile_pool(name="maskp", bufs=16 if masked else 1) as maskp,
                tc.tile_pool(name="osb", bufs=4) as osb,
            ):
                # PSUM: two score pair-spans (banks 0-3), single PV
                # accumulator (bank 4), denominator columns (bank 5),
                # projection/out-proj accumulators (banks 6-7).
                sp = [acc.tile([P, 2 * NQ], f32, name=f"sp{t}")
                      for t in range(2)]
                pvb = acc.tile([P, NQ], f32, name="pvb")
                dbk = acc.tile([P, NQ], f32, name="dbk")
                pj = [acc.tile([P, NQ], f32, name=f"pj{t}") for t in range(2)]

                # ---- DMA issue order == service order: V path first.
                for t, src in ((ident, ident_d), (bk_sb, bk_d),
                               (bv_sb, bvb_d), (bq_sb, bq_d),
                               (ones_col, ones_d)):
                    nc.sync.dma_start(t[:], src[:])
                # Warm-up: absorb the PE p-state ramp on 1-column matmuls
                # against the (tiny, first-arriving) identity tile.
                for _ in range(4):
                    nc.tensor.matmul(pj[1][:1, :1], ident[:, :1], ident[:, :1],
                                     start=True, stop=True)
                # Preload the Exp table while the PE projects.
                nc.scalar.activation(warm[:], bq_sb[:1, :], Exp)
                nc.sync.dma_start(wkb[:], wk_d[:])
                nc.sync.dma_start(eb[0][:], e_d[0][:])
                nc.sync.dma_start(wvb[:], wv_d[:])
                nc.sync.dma_start(eb[1][:], e_d[1][:])
                nc.sync.dma_start(wqb[:], wq_d[:])
                nc.sync.dma_start(xb[0][:], x_d[0][:])
                nc.sync.dma_start(eb[2][:], e_d[2][:])
                nc.sync.dma_start(eb[3][:], e_d[3][:])
                nc.sync.dma_start(xb[1][:], x_d[1][:])
                nc.sync.dma_start(wob[:], wo_d[:])

                # ---- projection bank-group emitters (banks 6-7 rotating)
                nbg = [0]

                def next_pj():
                    bank = pj[nbg[0] % 2]
                    nbg[0] += 1
                    return bank

                def kproj_group(h, lk):
                    bank = next_pj()
                    for d in range(KC):
                        nc.tensor.matmul(
                            bank[:],
                            wkb[:, d * OQ + h * DH:d * OQ + (h + 1) * DH],
                            eb[lk][:, d * NQ:(d + 1) * NQ],
                            start=(d == 0), stop=(d == KC - 1))
                    nc.vector.tensor_scalar_add(
                        kT[h][:, lk * NQ:(lk + 1) * NQ], bank[:],
                        bk_sb[:, h:h + 1])

                def qproj_group(h, q2):
                    bank = next_pj()
                    for d in range(KC):
                        nc.tensor.matmul(
                            bank[:],
                            wqb[:, d * OQ + h * DH:d * OQ + (h + 1) * DH],
                            xb[q2][:, d * NQ:(d + 1) * NQ],
                            start=(d == 0), stop=(d == KC - 1))
                    nc.vector.tensor_scalar_add(
                        qT[h][:, q2 * NQ:(q2 + 1) * NQ], bank[:],
                        bq_sb[:, h:h + 1])

                def vproj_group(j):
                    bank = next_pj()
                    for d in range(KC):
                        nc.tensor.matmul(
                            bank[:],
                            eb[j // 4][:, d * NQ + (j % 4) * P:
                                       d * NQ + (j % 4 + 1) * P],
                            wvb[:, d * OQ:(d + 1) * OQ],
                            start=(d == 0), stop=(d == KC - 1))
                    nc.vector.tensor_add(vch[j][:], bank[:], bv_sb[:])

                def oproj_group(lqc, o2, n):
                    bank = next_pj()
                    for h in range(HPC):
                        nc.tensor.matmul(
                            bank[:],
                            valsT[h][:, lqc * P:(lqc + 1) * P],
                            wob[:, h * D + o2 * NQ:h * D + (o2 + 1) * NQ],
                            start=(h == 0), stop=(h == HPC - 1))
                    ot = osb.tile([P, NQ], bf16, name="ot")
                    if n % 2 == 0:
                        nc.vector.tensor_copy(ot[:], bank[:])
                    else:
                        nc.scalar.activation(ot[:], bank[:], Copy)
                    nc.sync.dma_start(
                        out_d[lqc * P:(lqc + 1) * P,
                              o2 * NQ:(o2 + 1) * NQ], ot[:])

                # ---- attention emitters
                mask_tiles = [None] * LKC

                def attn_pair(q2, h, jp):
                    """Chunks j=2jp,2jp+1: scores into pair-span jp%2, one
                    exp over both, then PV + denominator matmuls."""
                    span = sp[jp % 2]
                    for t in range(2):
                        j = 2 * jp + t
                        nc.tensor.matmul(
                            span[:, t * NQ:(t + 1) * NQ],
                            kT[h][:, j * P:(j + 1) * P],
                            qT[h][:, q2 * NQ:(q2 + 1) * NQ],
                            start=True, stop=True)
                        if masked:
                            nc.vector.tensor_add(
                                span[:, t * NQ:(t + 1) * NQ],
                                span[:, t * NQ:(t + 1) * NQ],
                                mask_tiles[j][:])
                    pT2 = pTp.tile([P, 2 * NQ], bf16, name="pT2")
                    nc.scalar.activation(pT2[:], span[:], Exp)
                    for t in range(2):
                        j = 2 * jp + t
                        nc.tensor.matmul(
                            pvb[:],
                            vch[j][:, h * DH:(h + 1) * DH],
                            pT2[:, t * NQ:(t + 1) * NQ],
                            start=(j == 0), stop=(j == LKC - 1))
                        for s in range(4):
                            nc.tensor.matmul(
                                dbk[:, s:s + 1],
                                pT2[:, t * NQ + s * P:t * NQ + (s + 1) * P],
                                ones_col[:],
                                start=(j == 0 and s == 0),
                                stop=(j == LKC - 1 and s == 3),
                                skip_group_check=True)

                def attn_norm_start(q2, h):
                    """Free the PV/denominator banks: raw-copy the PV
                    accumulator and take the reciprocal of d."""
                    pvraw = smallp.tile([P, NQ], f32, name="pvraw")
                    nc.vector.tensor_copy(pvraw[:], pvb[:])
                    rsb = smallp.tile([P, 4], f32, name="rsb")
                    nc.vector.reciprocal(rsb[:], dbk[:, 0:4])
                    return pvraw, rsb

                def attn_norm_finish(q2, h, pvraw, rsb):
                    """1/d -> [1,512] row via PE transposes (into the dead
                    half of span B) -> partition broadcast -> scale."""
                    for s in range(4):
                        nc.tensor.transpose(
                            sp[1][0:1, NQ + s * P:NQ + (s + 1) * P],
                            rsb[:, s:s + 1], ident[:])
                    rrow = smallp.tile([1, NQ], f32, name="rrow")
                    nc.vector.tensor_copy(rrow[:], sp[1][0:1, NQ:])
                    bcast = smallp.tile([P, NQ], f32, name="bcast")
                    nc.gpsimd.partition_broadcast(bcast[:], rrow[:])
                    nc.vector.tensor_mul(
                        valsT[h][:, q2 * NQ:(q2 + 1) * NQ],
                        pvraw[:], bcast[:])

                # ---- the pipelined schedule.
                # Pre-stage: first K group, the V stream, then Q(h0,0).
                kproj_group(0, 0)
                for j in range(8):
                    vproj_group(j)
                qproj_group(0, 0)

                stage_work = {
                    0: [lambda: kproj_group(0, 1), lambda: vproj_group(8),
                        lambda: vproj_group(9), lambda: kproj_group(0, 2),
                        lambda: vproj_group(10), lambda: vproj_group(11),
                        lambda: kproj_group(0, 3), lambda: vproj_group(12),
                        lambda: vproj_group(13), lambda: kproj_group(1, 0),
                        lambda: vproj_group(14), lambda: vproj_group(15),
                        lambda: kproj_group(1, 1), lambda: kproj_group(1, 2),
                        lambda: kproj_group(1, 3), lambda: qproj_group(1, 0)],
                    1: [lambda: qproj_group(0, 1), lambda: qproj_group(1, 1),
                        lambda: kproj_group(2, 0), lambda: kproj_group(2, 1),
                        lambda: kproj_group(2, 2), lambda: kproj_group(2, 3),
                        lambda: qproj_group(2, 0), lambda: qproj_group(2, 1)],
                    2: [lambda: kproj_group(3, 0), lambda: kproj_group(3, 1),
                        lambda: kproj_group(3, 2), lambda: kproj_group(3, 3),
                        lambda: qproj_group(3, 0), lambda: qproj_group(3, 1)],
                }
                # out-proj q2=0: 8 groups spread 2 per stage over stages 4-7
                for st in range(4, 8):
                    k0 = 2 * (st - 4)
                    stage_work[st] = [
                        (lambda n=n: oproj_group(n // 2, n % 2, n))
                        for n in (k0, k0 + 1)
                    ]
                per_slot = {0: 2, 1: 1, 2: 1, 4: 1, 5: 1, 6: 1, 7: 1}

                pending = None
                for idx in range(8):
                    q2, h = idx // 4, idx % 4
                    if masked and h == 0:
                        for j in range(LKC):
                            mt = maskp.tile([P, NQ], f32, name=f"m{j}")
                            nc.sync.dma_start(
                                mt[:],
                                maskT[j * P:(j + 1) * P,
                                      q2 * NQ:(q2 + 1) * NQ])
                            mask_tiles[j] = mt
                    work = stage_work.get(idx, [])
                    rate = per_slot.get(idx, 1)
                    wi = 0
                    for jp in range(LKC // 2):
                        if idx < 4:
                            for _ in range(rate):
                                if wi < len(work):
                                    work[wi]()
                                    wi += 1
                            attn_pair(q2, h, jp)
                        else:
                            attn_pair(q2, h, jp)
                            if pending is None:
                                for _ in range(rate):
                                    if wi < len(work):
                                        work[wi]()
                                        wi += 1
                        if jp == 0 and pending is not None:
                            # finish the previous stage's normalize off the
                            # critical path
                            attn_norm_finish(*pending)
                            pending = None
                    while wi < len(work):
                        work[wi]()
                        wi += 1
                    pvraw, rsb = attn_norm_start(q2, h)
                    pending = (q2, h, pvraw, rsb)

                attn_norm_finish(*pending)
                # Tail: second-half output projection.
                for lqc in range(4, 8):
                    for o2 in range(2):
                        oproj_group(lqc, o2, 2 * lqc + o2)

    nc.compile()
    return nc


def _get_built(masked):
    if masked not in _BUILT:
        _BUILT[masked] = _build(masked)
    return _BUILT[masked]


def _shard_inputs(inputs, masked):
    import ml_dtypes

    bf16 = ml_dtypes.bfloat16

    x = np.asarray(inputs["mhca_input"], np.float32)
    enc = np.asarray(inputs["encoder_output"], np.float32)
    mask = np.asarray(inputs["cross_mask"], np.float32)
    W_kv = np.asarray(inputs["W_kv"], np.float32)
    b_kv = np.asarray(inputs["b_kv"], np.float32)
    W_q = np.asarray(inputs["W_q"], np.float32)
    b_q = np.asarray(inputs["b_q"], np.float32)
    W_o = np.asarray(inputs["W_o"], np.float32)

    scale = 1.0 / math.sqrt(DH)
    in_maps = []
    for c in range(N_CORES):
        b = c // 2
        g = c % 2
        heads = list(range(g * HPC, (g + 1) * HPC))
        sl = slice(g * OQ, (g + 1) * OQ)
        k_rows = np.concatenate(
            [W_kv[h * 2 * DH:h * 2 * DH + DH] for h in heads], 0)
        v_rows = np.concatenate(
            [W_kv[h * 2 * DH + DH:(h + 1) * 2 * DH] for h in heads], 0)
        bv_rows = np.concatenate(
            [b_kv[h * 2 * DH + DH:(h + 1) * 2 * DH] for h in heads], 0)
        xT = np.ascontiguousarray(x[b].T)      # [1024, 1024]
        encT = np.ascontiguousarray(enc[b].T)  # [1024, 2048]
        m = {
            "wk": _repack(k_rows.T).astype(bf16),
            "wv": _repack(v_rows.T).astype(bf16),
            "wq": _repack((W_q[sl] * scale).T).astype(bf16),
            "wo": _repack(W_o[:, sl].T).astype(bf16),
            "bq": np.ascontiguousarray((b_q[sl] * scale).reshape(HPC, DH).T),
            "bk": np.ascontiguousarray(
                np.stack([b_kv[h * 2 * DH:h * 2 * DH + DH] for h in heads], 1)),
            "bvb": np.ascontiguousarray(
                np.tile(bv_rows[None, :], (P, 1)).astype(np.float32)),
            "ones": np.ones((P, 1), bf16),
            "identT": np.eye(P, dtype=np.float32),
        }
        for q in range(4):
            m[f"enc{q}"] = _repack(encT[:, q * NQ:(q + 1) * NQ]).astype(bf16)
        for q in range(2):
            m[f"x{q}"] = _repack(xT[:, q * NQ:(q + 1) * NQ]).astype(bf16)
        if masked:
            m["maskT"] = np.ascontiguousarray(mask[b].T)
        in_maps.append(m)
    return in_maps


def kernel(mhca_input, encoder_output, cross_mask, W_kv, b_kv, W_q, b_q, W_o,
           b_o):
    from concourse.bass_utils import run_bass_kernel_spmd

    inputs = {
        "mhca_input": mhca_input, "encoder_output": encoder_output,
        "cross_mask": cross_mask, "W_kv": W_kv, "b_kv": b_kv, "W_q": W_q,
        "b_q": b_q, "W_o": W_o,
    }
    b_o = np.asarray(b_o, np.float32)
    masked = bool(np.any(np.asarray(cross_mask)))
    nc = _get_built(masked)
    in_maps = _shard_inputs(inputs, masked)

    res = run_bass_kernel_spmd(nc, in_maps, core_ids=list(range(N_CORES)))
    outs = [np.asarray(res.results[c]["out"], np.float32)
            for c in range(N_CORES)]
    full = np.stack([outs[2 * b] + outs[2 * b + 1] for b in range(B)], 0)
    return (full + b_o[None, None, :]).astype(np.float32)


# revision 16
# speedup vs baseline: 1.2816x; 1.0174x over previous
"""Multi-head cross-attention on 8 Trainium2 NeuronCores.

Problem shapes (hardcoded): B=4, Ld=1024, Le=2048, d_model=1024, 8 heads x 128.
Sharding: core c handles batch b=c//2 and head-group g=c%2 (4 heads each).
Each core computes q/k/v projections for its heads, attention, and a partial
output projection over its heads' value dims; the host sums the two partial
outputs per batch and adds b_o.

All inputs are host-repacked into contiguous [128, X] layouts so each tensor
is a single large DMA (descriptor-generation slots are an exclusive 625ns
resource; dozens of small DMAs serialize the front of the kernel).

Schedule: one software-pipelined stream.  Projection bank-groups (K/Q/V and
later the output projection) are queued and drained a few per attention
chunk-pair, so the PE never waits for the Act engine's exp stream and the
whole kernel is PE-bound.  DMA order feeds the V path first (wk, enc, wv),
giving the PE continuous early work while wq/x stream in.

Softmax denominators use tall-skinny matmuls: pT is the *stationary* operand
and a ones column moves, so each [128,1] per-query partial sum costs ~1 PE
row instead of the 512 rows a [1,512] ones-stationary layout costs.

Exps are issued per chunk-pair over a [128,1024] two-bank PSUM span (halving
the Act engine's fixed access overhead), double-buffered across two spans so
scores never wait on the previous exp.

The per-stage normalize chain (reciprocal -> PE-transpose to a [1,512] row
-> gpsimd partition-broadcast -> multiply) is split: the PV accumulator is
drained to SBUF raw (one DVE copy) so the next stage's PV can start
immediately, and the rest of the chain is emitted after the next stage's
first pair, fully off the PE's critical path.

PSUM banks: 0-3 two score pair-spans, 4 PV accumulator, 5 denominator
columns, 6-7 projection / output-projection accumulators (rotating).
"""

import math
import sys

import numpy as np

for _p in ("/opt/trn_rl_repo", "/root/.axon_site/_ro/trn_rl_repo"):
    if _p not in sys.path:
        sys.path.append(_p)

B = 4
LQ = 1024
LK = 2048
D = 1024
H = 8
DH = 128
P = 128
HPC = 4          # heads per core
OQ = HPC * DH    # 512 projected dims per core
NQ = 512         # matmul moving free dim
KC = D // P      # 8 contraction chunks for projections
LKC = LK // P    # 16 key chunks
N_CORES = 8

_BUILT = {}


def _repack(a):
    """[KC*128, X] -> [128, KC*X] with d-chunk-major columns."""
    kc = a.shape[0] // P
    return np.ascontiguousarray(
        a.reshape(kc, P, a.shape[1]).transpose(1, 0, 2).reshape(P, -1))


def _build(masked):
    import concourse.bass as bass  # noqa: F401
    import concourse.tile as tile
    import concourse.mybir as mybir
    from concourse import bacc

    f32 = mybir.dt.float32
    bf16 = mybir.dt.bfloat16
    f32r = mybir.dt.float32r
    Exp = mybir.ActivationFunctionType.Exp
    Copy = mybir.ActivationFunctionType.Copy

    nc = bacc.Bacc("TRN2", target_bir_lowering=False, debug=False,
                   num_devices=N_CORES)

    BW = KC * NQ  # 4096: big packed width
    x_d = [nc.dram_tensor(f"x{q}", [P, BW], bf16, kind="ExternalInput").ap()
           for q in range(2)]
    e_d = [nc.dram_tensor(f"enc{q}", [P, BW], bf16, kind="ExternalInput").ap()
           for q in range(4)]
    wk_d = nc.dram_tensor("wk", [P, BW], bf16, kind="ExternalInput").ap()
    wv_d = nc.dram_tensor("wv", [P, BW], bf16, kind="ExternalInput").ap()
    wq_d = nc.dram_tensor("wq", [P, BW], bf16, kind="ExternalInput").ap()
    wo_d = nc.dram_tensor("wo", [P, BW], bf16, kind="ExternalInput").ap()
    bq_d = nc.dram_tensor("bq", [P, HPC], f32, kind="ExternalInput").ap()
    bk_d = nc.dram_tensor("bk", [P, HPC], f32, kind="ExternalInput").ap()
    bvb_d = nc.dram_tensor("bvb", [P, OQ], f32, kind="ExternalInput").ap()
    ones_d = nc.dram_tensor("ones", [P, 1], bf16, kind="ExternalInput").ap()
    ident_d = nc.dram_tensor("identT", [P, P], f32, kind="ExternalInput").ap()
    if masked:
        maskT = nc.dram_tensor("maskT", [LK, LQ], f32, kind="ExternalInput").ap()
    out_d = nc.dram_tensor("out", [LQ, D], bf16, kind="ExternalOutput").ap()

    with tile.TileContext(nc) as tc:
        with tc.tile_pool(name="persist", bufs=1) as persist:
            qT = [persist.tile([P, LQ], f32r, name=f"qT{h}") for h in range(HPC)]
            kT = [persist.tile([P, LK], f32r, name=f"kT{h}") for h in range(HPC)]
            vch = [persist.tile([P, OQ], bf16, name=f"v{j}") for j in range(LKC)]
            bq_sb = persist.tile([P, HPC], f32, name="bq")
            bk_sb = persist.tile([P, HPC], f32, name="bk")
            bv_sb = persist.tile([P, OQ], f32, name="bvb")
            ones_col = persist.tile([P, 1], bf16, name="ones")
            ident = persist.tile([P, P], f32, name="ident")
            warm = persist.tile([1, HPC], f32, name="warm")
            wkb = persist.tile([P, BW], bf16, name="wkb")
            wvb = persist.tile([P, BW], bf16, name="wvb")
            wqb = persist.tile([P, BW], bf16, name="wqb")
            wob = persist.tile([P, BW], bf16, name="wob")
            eb = [persist.tile([P, BW], bf16, name=f"eb{q}") for q in range(4)]
            xb = [persist.tile([P, BW], bf16, name=f"xb{q}") for q in range(2)]
            valsT = [persist.tile([P, LQ], bf16, name=f"valsT{h}")
                     for h in range(HPC)]

            with (
                tc.tile_pool(name="acc", bufs=1, space="PSUM") as acc,
                tc.tile_pool(name="pTp", bufs=3) as pTp,
                tc.tile_pool(name="smallp", bufs=2) as smallp,
                tc.tile_pool(name="maskp", bufs=16 if masked else 1) as maskp,
                tc.tile_pool(name="osb", bufs=4) as osb,
            ):
                # PSUM: two score pair-spans (banks 0-3), single PV
                # accumulator (bank 4), denominator columns (bank 5),
                # projection/out-proj accumulators (banks 6-7).
                sp = [acc.tile([P, 2 * NQ], f32, name=f"sp{t}")
                      for t in range(2)]
                pvb = acc.tile([P, NQ], f32, name="pvb")
                dbk = acc.tile([P, NQ], f32, name="dbk")
                pj = [acc.tile([P, NQ], f32, name=f"pj{t}") for t in range(2)]

                # Warm-up: absorb the PE p-state ramp on 1-column matmuls
                # against a memset tile (no DMA dependency), and preload the
                # Act Exp table the same way.
                wt = persist.tile([P, 4], bf16, name="wt")
                nc.vector.memset(wt[:], 1.0)
                for _ in range(4):
                    nc.tensor.matmul(pj[1][:1, :1], wt[:, :1], wt[:, :1],
                                     start=True, stop=True)
                nc.scalar.activation(warm[:], wt[:1, :], Exp)
                # ---- DMA issue order == service order: V path first, and
                # the first K-group's tensors split in half for an earlier
                # start. Small tensors ride in the first gap they're needed.
                hw_ = BW // 2
                nc.sync.dma_start(wkb[:, :hw_], wk_d[:, :hw_])
                nc.sync.dma_start(eb[0][:, :hw_], e_d[0][:, :hw_])
                nc.sync.dma_start(wkb[:, hw_:], wk_d[:, hw_:])
                nc.sync.dma_start(eb[0][:, hw_:], e_d[0][:, hw_:])
                for t, src in ((bk_sb, bk_d), (bv_sb, bvb_d), (bq_sb, bq_d),
                               (ones_col, ones_d), (ident, ident_d)):
                    nc.sync.dma_start(t[:], src[:])
                nc.sync.dma_start(wvb[:], wv_d[:])
                nc.sync.dma_start(eb[1][:], e_d[1][:])
                nc.sync.dma_start(wqb[:], wq_d[:])
                nc.sync.dma_start(xb[0][:], x_d[0][:])
                nc.sync.dma_start(eb[2][:], e_d[2][:])
                nc.sync.dma_start(eb[3][:], e_d[3][:])
                nc.sync.dma_start(xb[1][:], x_d[1][:])
                nc.sync.dma_start(wob[:], wo_d[:])

                # ---- projection bank-group emitters (banks 6-7 rotating)
                nbg = [0]

                def next_pj():
                    bank = pj[nbg[0] % 2]
                    nbg[0] += 1
                    return bank

                def kproj_group(h, lk):
                    bank = next_pj()
                    for d in range(KC):
                        nc.tensor.matmul(
                            bank[:],
                            wkb[:, d * OQ + h * DH:d * OQ + (h + 1) * DH],
                            eb[lk][:, d * NQ:(d + 1) * NQ],
                            start=(d == 0), stop=(d == KC - 1))
                    nc.vector.tensor_scalar_add(
                        kT[h][:, lk * NQ:(lk + 1) * NQ], bank[:],
                        bk_sb[:, h:h + 1])

                def qproj_group(h, q2):
                    bank = next_pj()
                    for d in range(KC):
                        nc.tensor.matmul(
                            bank[:],
                            wqb[:, d * OQ + h * DH:d * OQ + (h + 1) * DH],
                            xb[q2][:, d * NQ:(d + 1) * NQ],
                            start=(d == 0), stop=(d == KC - 1))
                    nc.vector.tensor_scalar_add(
                        qT[h][:, q2 * NQ:(q2 + 1) * NQ], bank[:],
                        bq_sb[:, h:h + 1])

                def vproj_group(j):
                    bank = next_pj()
                    for d in range(KC):
                        nc.tensor.matmul(
                            bank[:],
                            eb[j // 4][:, d * NQ + (j % 4) * P:
                                       d * NQ + (j % 4 + 1) * P],
                            wvb[:, d * OQ:(d + 1) * OQ],
                            start=(d == 0), stop=(d == KC - 1))
                    nc.vector.tensor_add(vch[j][:], bank[:], bv_sb[:])

                def oproj_group(lqc, o2, n):
                    bank = next_pj()
                    for h in range(HPC):
                        nc.tensor.matmul(
                            bank[:],
                            valsT[h][:, lqc * P:(lqc + 1) * P],
                            wob[:, h * D + o2 * NQ:h * D + (o2 + 1) * NQ],
                            start=(h == 0), stop=(h == HPC - 1))
                    ot = osb.tile([P, NQ], bf16, name="ot")
                    if n % 2 == 0:
                        nc.vector.tensor_copy(ot[:], bank[:])
                    else:
                        nc.scalar.activation(ot[:], bank[:], Copy)
                    nc.sync.dma_start(
                        out_d[lqc * P:(lqc + 1) * P,
                              o2 * NQ:(o2 + 1) * NQ], ot[:])

                # ---- attention emitters
                mask_tiles = [None] * LKC

                def attn_pair(q2, h, jp):
                    """Chunks j=2jp,2jp+1: scores into pair-span jp%2, one
                    exp over both, then PV + denominator matmuls."""
                    span = sp[jp % 2]
                    for t in range(2):
                        j = 2 * jp + t
                        nc.tensor.matmul(
                            span[:, t * NQ:(t + 1) * NQ],
                            kT[h][:, j * P:(j + 1) * P],
                            qT[h][:, q2 * NQ:(q2 + 1) * NQ],
                            start=True, stop=True)
                        if masked:
                            nc.vector.tensor_add(
                                span[:, t * NQ:(t + 1) * NQ],
                                span[:, t * NQ:(t + 1) * NQ],
                                mask_tiles[j][:])
                    pT2 = pTp.tile([P, 2 * NQ], bf16, name="pT2")
                    nc.scalar.activation(pT2[:], span[:], Exp)
                    for t in range(2):
                        j = 2 * jp + t
                        nc.tensor.matmul(
                            pvb[:],
                            vch[j][:, h * DH:(h + 1) * DH],
                            pT2[:, t * NQ:(t + 1) * NQ],
                            start=(j == 0), stop=(j == LKC - 1))
                        for s in range(4):
                            nc.tensor.matmul(
                                dbk[:, s:s + 1],
                                pT2[:, t * NQ + s * P:t * NQ + (s + 1) * P],
                                ones_col[:],
                                start=(j == 0 and s == 0),
                                stop=(j == LKC - 1 and s == 3),
                                skip_group_check=True)

                def attn_norm_start(q2, h):
                    """Free the PV/denominator banks: raw-copy the PV
                    accumulator and take the reciprocal of d."""
                    pvraw = smallp.tile([P, NQ], f32, name="pvraw")
                    nc.vector.tensor_copy(pvraw[:], pvb[:])
                    rsb = smallp.tile([P, 4], f32, name="rsb")
                    nc.vector.reciprocal(rsb[:], dbk[:, 0:4])
                    return pvraw, rsb

                def attn_norm_finish(q2, h, pvraw, rsb):
                    """1/d -> [1,512] row via PE transposes (into the dead
                    half of span B) -> partition broadcast -> scale."""
                    for s in range(4):
                        nc.tensor.transpose(
                            sp[1][0:1, NQ + s * P:NQ + (s + 1) * P],
                            rsb[:, s:s + 1], ident[:])
                    rrow = smallp.tile([1, NQ], f32, name="rrow")
                    nc.vector.tensor_copy(rrow[:], sp[1][0:1, NQ:])
                    bcast = smallp.tile([P, NQ], f32, name="bcast")
                    nc.gpsimd.partition_broadcast(bcast[:], rrow[:])
                    nc.vector.tensor_mul(
                        valsT[h][:, q2 * NQ:(q2 + 1) * NQ],
                        pvraw[:], bcast[:])

                # ---- the pipelined schedule.
                # Pre-stage: first K group, the V stream, then Q(h0,0).
                kproj_group(0, 0)
                for j in range(8):
                    vproj_group(j)
                qproj_group(0, 0)

                stage_work = {
                    0: [lambda: kproj_group(0, 1), lambda: vproj_group(8),
                        lambda: vproj_group(9), lambda: kproj_group(0, 2),
                        lambda: vproj_group(10), lambda: vproj_group(11),
                        lambda: kproj_group(0, 3), lambda: vproj_group(12),
                        lambda: vproj_group(13), lambda: kproj_group(1, 0),
                        lambda: vproj_group(14), lambda: vproj_group(15),
                        lambda: kproj_group(1, 1), lambda: kproj_group(1, 2),
                        lambda: kproj_group(1, 3), lambda: qproj_group(1, 0)],
                    1: [lambda: qproj_group(0, 1), lambda: qproj_group(1, 1),
                        lambda: kproj_group(2, 0), lambda: kproj_group(2, 1),
                        lambda: kproj_group(2, 2), lambda: kproj_group(2, 3),
                        lambda: qproj_group(2, 0), lambda: qproj_group(2, 1)],
                    2: [lambda: kproj_group(3, 0), lambda: kproj_group(3, 1),
                        lambda: kproj_group(3, 2), lambda: kproj_group(3, 3),
                        lambda: qproj_group(3, 0), lambda: qproj_group(3, 1)],
                }
                # out-proj q2=0: 8 groups spread 2 per stage over stages 4-7
                for st in range(4, 8):
                    k0 = 2 * (st - 4)
                    stage_work[st] = [
                        (lambda n=n: oproj_group(n // 2, n % 2, n))
                        for n in (k0, k0 + 1)
                    ]
                per_slot = {0: 2, 1: 1, 2: 1, 4: 1, 5: 1, 6: 1, 7: 1}

                pending = None
                for idx in range(8):
                    q2, h = idx // 4, idx % 4
                    if masked and h == 0:
                        for j in range(LKC):
                            mt = maskp.tile([P, NQ], f32, name=f"m{j}")
                            nc.sync.dma_start(
                                mt[:],
                                maskT[j * P:(j + 1) * P,
                                      q2 * NQ:(q2 + 1) * NQ])
                            mask_tiles[j] = mt
                    work = stage_work.get(idx, [])
                    rate = per_slot.get(idx, 1)
                    wi = 0
                    for jp in range(LKC // 2):
                        if idx < 4:
                            for _ in range(rate):
                                if wi < len(work):
                                    work[wi]()
                                    wi += 1
                            attn_pair(q2, h, jp)
                        else:
                            attn_pair(q2, h, jp)
                            # out-proj trails by a few pairs so the previous
                            # stage's normalize chain has fully landed in
                            # valsT before the PE reads it
                            if jp >= 3:
                                for _ in range(rate):
                                    if wi < len(work):
                                        work[wi]()
                                        wi += 1
                        if jp == 0 and pending is not None:
                            # finish the previous stage's normalize off the
                            # critical path
                            attn_norm_finish(*pending)
                            pending = None
                    while wi < len(work):
                        work[wi]()
                        wi += 1
                    pvraw, rsb = attn_norm_start(q2, h)
                    pending = (q2, h, pvraw, rsb)

                attn_norm_finish(*pending)
                # Tail: second-half output projection.
                for lqc in range(4, 8):
                    for o2 in range(2):
                        oproj_group(lqc, o2, 2 * lqc + o2)

    nc.compile()
    return nc


def _get_built(masked):
    if masked not in _BUILT:
        _BUILT[masked] = _build(masked)
    return _BUILT[masked]


def _shard_inputs(inputs, masked):
    import ml_dtypes

    bf16 = ml_dtypes.bfloat16

    x = np.asarray(inputs["mhca_input"], np.float32)
    enc = np.asarray(inputs["encoder_output"], np.float32)
    mask = np.asarray(inputs["cross_mask"], np.float32)
    W_kv = np.asarray(inputs["W_kv"], np.float32)
    b_kv = np.asarray(inputs["b_kv"], np.float32)
    W_q = np.asarray(inputs["W_q"], np.float32)
    b_q = np.asarray(inputs["b_q"], np.float32)
    W_o = np.asarray(inputs["W_o"], np.float32)

    scale = 1.0 / math.sqrt(DH)
    in_maps = []
    for c in range(N_CORES):
        b = c // 2
        g = c % 2
        heads = list(range(g * HPC, (g + 1) * HPC))
        sl = slice(g * OQ, (g + 1) * OQ)
        k_rows = np.concatenate(
            [W_kv[h * 2 * DH:h * 2 * DH + DH] for h in heads], 0)
        v_rows = np.concatenate(
            [W_kv[h * 2 * DH + DH:(h + 1) * 2 * DH] for h in heads], 0)
        bv_rows = np.concatenate(
            [b_kv[h * 2 * DH + DH:(h + 1) * 2 * DH] for h in heads], 0)
        xT = np.ascontiguousarray(x[b].T)      # [1024, 1024]
        encT = np.ascontiguousarray(enc[b].T)  # [1024, 2048]
        m = {
            "wk": _repack(k_rows.T).astype(bf16),
            "wv": _repack(v_rows.T).astype(bf16),
            "wq": _repack((W_q[sl] * scale).T).astype(bf16),
            "wo": _repack(W_o[:, sl].T).astype(bf16),
            "bq": np.ascontiguousarray((b_q[sl] * scale).reshape(HPC, DH).T),
            "bk": np.ascontiguousarray(
                np.stack([b_kv[h * 2 * DH:h * 2 * DH + DH] for h in heads], 1)),
            "bvb": np.ascontiguousarray(
                np.tile(bv_rows[None, :], (P, 1)).astype(np.float32)),
            "ones": np.ones((P, 1), bf16),
            "identT": np.eye(P, dtype=np.float32),
        }
        for q in range(4):
            m[f"enc{q}"] = _repack(encT[:, q * NQ:(q + 1) * NQ]).astype(bf16)
        for q in range(2):
            m[f"x{q}"] = _repack(xT[:, q * NQ:(q + 1) * NQ]).astype(bf16)
        if masked:
            m["maskT"] = np.ascontiguousarray(mask[b].T)
        in_maps.append(m)
    return in_maps


def kernel(mhca_input, encoder_output, cross_mask, W_kv, b_kv, W_q, b_q, W_o,
           b_o):
    from concourse.bass_utils import run_bass_kernel_spmd

    inputs = {
        "mhca_input": mhca_input, "encoder_output": encoder_output,
        "cross_mask": cross_mask, "W_kv": W_kv, "b_kv": b_kv, "W_q": W_q,
        "b_q": b_q, "W_o": W_o,
    }
    b_o = np.asarray(b_o, np.float32)
    masked = bool(np.any(np.asarray(cross_mask)))
    nc = _get_built(masked)
    in_maps = _shard_inputs(inputs, masked)

    res = run_bass_kernel_spmd(nc, in_maps, core_ids=list(range(N_CORES)))
    outs = [np.asarray(res.results[c]["out"], np.float32)
            for c in range(N_CORES)]
    full = np.stack([outs[2 * b] + outs[2 * b + 1] for b in range(B)], 0)
    return (full + b_o[None, None, :]).astype(np.float32)


# revision 20
# speedup vs baseline: 1.3109x; 1.0228x over previous
"""Multi-head cross-attention on 8 Trainium2 NeuronCores.

Problem shapes (hardcoded): B=4, Ld=1024, Le=2048, d_model=1024, 8 heads x 128.
Sharding: core c handles batch b=c//2 and head-group g=c%2 (4 heads each).
Each core computes q/k/v projections for its heads, attention, and a partial
output projection over its heads' value dims; the host sums the two partial
outputs per batch and adds b_o.

All inputs are host-repacked into contiguous [128, X] layouts so each tensor
is a single large DMA (descriptor-generation slots are an exclusive 625ns
resource; dozens of small DMAs serialize the front of the kernel).

Schedule: one software-pipelined stream.  Projection bank-groups (K/Q/V and
later the output projection) are queued and drained a few per attention
chunk-pair, so the PE never waits for the Act engine's exp stream and the
whole kernel is PE-bound.  DMA order feeds the V path first (wk, enc, wv),
giving the PE continuous early work while wq/x stream in.

Softmax denominators use tall-skinny matmuls: pT is the *stationary* operand
and a ones column moves, so each [128,1] per-query partial sum costs ~1 PE
row instead of the 512 rows a [1,512] ones-stationary layout costs.

Exps are issued per chunk-pair over a [128,1024] two-bank PSUM span (halving
the Act engine's fixed access overhead), double-buffered across two spans so
scores never wait on the previous exp.

The per-stage normalize chain (reciprocal -> PE-transpose to a [1,512] row
-> gpsimd partition-broadcast -> multiply) is split: the PV accumulator is
drained to SBUF raw (one DVE copy) so the next stage's PV can start
immediately, and the rest of the chain is emitted after the next stage's
first pair, fully off the PE's critical path.

PSUM banks: 0-3 two score pair-spans, 4 PV accumulator, 5 denominator
columns, 6-7 projection / output-projection accumulators (rotating).
"""

import math
import sys

import numpy as np

for _p in ("/opt/trn_rl_repo", "/root/.axon_site/_ro/trn_rl_repo"):
    if _p not in sys.path:
        sys.path.append(_p)

B = 4
LQ = 1024
LK = 2048
D = 1024
H = 8
DH = 128
P = 128
HPC = 4          # heads per core
OQ = HPC * DH    # 512 projected dims per core
NQ = 512         # matmul moving free dim
KC = D // P      # 8 contraction chunks for projections
LKC = LK // P    # 16 key chunks
N_CORES = 8

_BUILT = {}


def _repack(a):
    """[KC*128, X] -> [128, KC*X] with d-chunk-major columns."""
    kc = a.shape[0] // P
    return np.ascontiguousarray(
        a.reshape(kc, P, a.shape[1]).transpose(1, 0, 2).reshape(P, -1))


def _build(masked):
    import concourse.bass as bass  # noqa: F401
    import concourse.tile as tile
    import concourse.mybir as mybir
    from concourse import bacc

    f32 = mybir.dt.float32
    bf16 = mybir.dt.bfloat16
    f32r = mybir.dt.float32r
    Exp = mybir.ActivationFunctionType.Exp
    Copy = mybir.ActivationFunctionType.Copy

    nc = bacc.Bacc("TRN2", target_bir_lowering=False, debug=False,
                   num_devices=N_CORES)

    BW = KC * NQ  # 4096: big packed width
    x_d = [nc.dram_tensor(f"x{q}", [P, BW], bf16, kind="ExternalInput").ap()
           for q in range(2)]
    e_d = [nc.dram_tensor(f"enc{q}", [P, BW], bf16, kind="ExternalInput").ap()
           for q in range(4)]
    wk_d = nc.dram_tensor("wk", [P, BW], bf16, kind="ExternalInput").ap()
    wv_d = nc.dram_tensor("wv", [P, BW], bf16, kind="ExternalInput").ap()
    wq_d = nc.dram_tensor("wq", [P, BW], bf16, kind="ExternalInput").ap()
    wo_d = nc.dram_tensor("wo", [P, BW], bf16, kind="ExternalInput").ap()
    # bk | bq | bvb | ident packed as one f32 DMA (cols 0:4, 4:8, 8:520,
    # 520:648); ones is bf16 and rides separately.
    smf_d = nc.dram_tensor("smf", [P, 648], f32, kind="ExternalInput").ap()
    ones_d = nc.dram_tensor("ones", [P, 1], bf16, kind="ExternalInput").ap()
    if masked:
        maskT = nc.dram_tensor("maskT", [LK, LQ], f32, kind="ExternalInput").ap()
    out_d = nc.dram_tensor("out", [LQ, D], bf16, kind="ExternalOutput").ap()

    with tile.TileContext(nc) as tc:
        with tc.tile_pool(name="persist", bufs=1) as persist:
            qT = [persist.tile([P, LQ], f32r, name=f"qT{h}") for h in range(HPC)]
            kT = [persist.tile([P, LK], f32r, name=f"kT{h}") for h in range(HPC)]
            vch = [persist.tile([P, OQ], bf16, name=f"v{j}") for j in range(LKC)]
            smf = persist.tile([P, 648], f32, name="smf")
            bk_sb = smf[:, 0:HPC]
            bq_sb = smf[:, HPC:2 * HPC]
            bv_sb = smf[:, 2 * HPC:2 * HPC + OQ]
            ident = smf[:, 2 * HPC + OQ:2 * HPC + OQ + P]
            ones_col = persist.tile([P, 1], bf16, name="ones")
            warm = persist.tile([1, HPC], f32, name="warm")
            wkb = persist.tile([P, BW], bf16, name="wkb")
            wvb = persist.tile([P, BW], bf16, name="wvb")
            wqb = persist.tile([P, BW], bf16, name="wqb")
            wob = persist.tile([P, BW], bf16, name="wob")
            eb = [persist.tile([P, BW], bf16, name=f"eb{q}") for q in range(4)]
            xb = [persist.tile([P, BW], bf16, name=f"xb{q}") for q in range(2)]
            valsT = [persist.tile([P, LQ], bf16, name=f"valsT{h}")
                     for h in range(HPC)]

            with (
                tc.tile_pool(name="acc", bufs=1, space="PSUM") as acc,
                tc.tile_pool(name="pTp", bufs=3) as pTp,
                tc.tile_pool(name="smallp", bufs=2) as smallp,
                tc.tile_pool(name="maskp", bufs=16 if masked else 1) as maskp,
                tc.tile_pool(name="osb", bufs=4) as osb,
            ):
                # PSUM: two score pair-spans (banks 0-3), single PV
                # accumulator (bank 4), denominator columns (bank 5),
                # projection/out-proj accumulators (banks 6-7).
                sp = [acc.tile([P, 2 * NQ], f32, name=f"sp{t}")
                      for t in range(2)]
                pvb = acc.tile([P, NQ], f32, name="pvb")
                dbk = acc.tile([P, NQ], f32, name="dbk")
                pj = [acc.tile([P, NQ], f32, name=f"pj{t}") for t in range(2)]

                # Warm-up: absorb the PE p-state ramp on 1-column matmuls
                # against a memset tile (no DMA dependency), and preload the
                # Act Exp table the same way.
                wt = persist.tile([P, 4], bf16, name="wt")
                nc.vector.memset(wt[:], 1.0)
                for _ in range(4):
                    nc.tensor.matmul(pj[1][:1, :1], wt[:, :1], wt[:, :1],
                                     start=True, stop=True)
                nc.scalar.activation(warm[:], wt[:1, :], Exp)
                # ---- DMA issue order == service order: V path first, and
                # the first K-group's tensors split in half for an earlier
                # start. Small tensors ride in the first gap they're needed.
                hw_ = BW // 2
                nc.sync.dma_start(wkb[:, :hw_], wk_d[:, :hw_])
                nc.sync.dma_start(eb[0][:, :hw_], e_d[0][:, :hw_])
                nc.sync.dma_start(wkb[:, hw_:], wk_d[:, hw_:])
                nc.sync.dma_start(eb[0][:, hw_:], e_d[0][:, hw_:])
                nc.sync.dma_start(smf[:], smf_d[:])
                nc.sync.dma_start(ones_col[:], ones_d[:])
                nc.sync.dma_start(wvb[:], wv_d[:])
                nc.sync.dma_start(eb[1][:], e_d[1][:])
                nc.sync.dma_start(wqb[:], wq_d[:])
                nc.sync.dma_start(xb[0][:], x_d[0][:])
                nc.sync.dma_start(eb[2][:], e_d[2][:])
                nc.sync.dma_start(eb[3][:], e_d[3][:])
                nc.sync.dma_start(xb[1][:], x_d[1][:])
                nc.sync.dma_start(wob[:], wo_d[:])

                # ---- projection bank-group emitters (banks 6-7 rotating)
                nbg = [0]

                def next_pj():
                    bank = pj[nbg[0] % 2]
                    nbg[0] += 1
                    return bank

                def kproj_group(h, lk):
                    bank = next_pj()
                    for d in range(KC):
                        nc.tensor.matmul(
                            bank[:],
                            wkb[:, d * OQ + h * DH:d * OQ + (h + 1) * DH],
                            eb[lk][:, d * NQ:(d + 1) * NQ],
                            start=(d == 0), stop=(d == KC - 1))
                    nc.vector.tensor_scalar_add(
                        kT[h][:, lk * NQ:(lk + 1) * NQ], bank[:],
                        bk_sb[:, h:h + 1])

                def qproj_group(h, q2):
                    bank = next_pj()
                    for d in range(KC):
                        nc.tensor.matmul(
                            bank[:],
                            wqb[:, d * OQ + h * DH:d * OQ + (h + 1) * DH],
                            xb[q2][:, d * NQ:(d + 1) * NQ],
                            start=(d == 0), stop=(d == KC - 1))
                    nc.vector.tensor_scalar_add(
                        qT[h][:, q2 * NQ:(q2 + 1) * NQ], bank[:],
                        bq_sb[:, h:h + 1])

                def vproj_group(j):
                    bank = next_pj()
                    for d in range(KC):
                        nc.tensor.matmul(
                            bank[:],
                            eb[j // 4][:, d * NQ + (j % 4) * P:
                                       d * NQ + (j % 4 + 1) * P],
                            wvb[:, d * OQ:(d + 1) * OQ],
                            start=(d == 0), stop=(d == KC - 1))
                    nc.vector.tensor_add(vch[j][:], bank[:], bv_sb[:])

                def oproj_group(lqc, o2, n):
                    bank = next_pj()
                    for h in range(HPC):
                        nc.tensor.matmul(
                            bank[:],
                            valsT[h][:, lqc * P:(lqc + 1) * P],
                            wob[:, h * D + o2 * NQ:h * D + (o2 + 1) * NQ],
                            start=(h == 0), stop=(h == HPC - 1))
                    ot = osb.tile([P, NQ], bf16, name="ot")
                    if n % 2 == 0:
                        nc.vector.tensor_copy(ot[:], bank[:])
                    else:
                        nc.scalar.activation(ot[:], bank[:], Copy)
                    nc.sync.dma_start(
                        out_d[lqc * P:(lqc + 1) * P,
                              o2 * NQ:(o2 + 1) * NQ], ot[:])

                # ---- attention emitters
                mask_tiles = [None] * LKC

                def attn_pair(q2, h, jp):
                    """Chunks j=2jp,2jp+1: scores into pair-span jp%2, one
                    exp over both, then PV + denominator matmuls."""
                    span = sp[jp % 2]
                    for t in range(2):
                        j = 2 * jp + t
                        nc.tensor.matmul(
                            span[:, t * NQ:(t + 1) * NQ],
                            kT[h][:, j * P:(j + 1) * P],
                            qT[h][:, q2 * NQ:(q2 + 1) * NQ],
                            start=True, stop=True)
                        if masked:
                            nc.vector.tensor_add(
                                span[:, t * NQ:(t + 1) * NQ],
                                span[:, t * NQ:(t + 1) * NQ],
                                mask_tiles[j][:])
                    pT2 = pTp.tile([P, 2 * NQ], bf16, name="pT2")
                    nc.scalar.activation(pT2[:], span[:], Exp)
                    for t in range(2):
                        j = 2 * jp + t
                        nc.tensor.matmul(
                            pvb[:],
                            vch[j][:, h * DH:(h + 1) * DH],
                            pT2[:, t * NQ:(t + 1) * NQ],
                            start=(j == 0), stop=(j == LKC - 1))
                        for s in range(4):
                            nc.tensor.matmul(
                                dbk[:, s:s + 1],
                                pT2[:, t * NQ + s * P:t * NQ + (s + 1) * P],
                                ones_col[:],
                                start=(j == 0 and s == 0),
                                stop=(j == LKC - 1 and s == 3),
                                skip_group_check=True)

                def attn_norm_start(q2, h):
                    """Free the PV/denominator banks: raw-copy the PV
                    accumulator and take the reciprocal of d."""
                    pvraw = smallp.tile([P, NQ], f32, name="pvraw")
                    nc.vector.tensor_copy(pvraw[:], pvb[:])
                    rsb = smallp.tile([P, 4], f32, name="rsb")
                    nc.vector.reciprocal(rsb[:], dbk[:, 0:4])
                    return pvraw, rsb

                def attn_norm_finish(q2, h, pvraw, rsb):
                    """1/d -> [1,512] row via PE transposes (into the dead
                    half of span B) -> partition broadcast -> scale."""
                    for s in range(4):
                        nc.tensor.transpose(
                            sp[1][0:1, NQ + s * P:NQ + (s + 1) * P],
                            rsb[:, s:s + 1], ident[:])
                    rrow = smallp.tile([1, NQ], f32, name="rrow")
                    nc.vector.tensor_copy(rrow[:], sp[1][0:1, NQ:])
                    bcast = smallp.tile([P, NQ], f32, name="bcast")
                    nc.gpsimd.partition_broadcast(bcast[:], rrow[:])
                    nc.vector.tensor_mul(
                        valsT[h][:, q2 * NQ:(q2 + 1) * NQ],
                        pvraw[:], bcast[:])

                # ---- the pipelined schedule.
                # Pre-stage: first K group, the V stream, then Q(h0,0).
                kproj_group(0, 0)
                for j in range(8):
                    vproj_group(j)
                qproj_group(0, 0)

                K, Q, V = kproj_group, qproj_group, vproj_group

                def O(n):
                    return lambda: oproj_group(n // 2, n % 2, n)

                def L(f, *a):
                    return lambda: f(*a)

                # Per-stage, per-pair-slot work placement.  Projections lead
                # the attention that consumes them; out-proj groups go at
                # jp>=5 of their earliest stage or jp=0 of later stages so
                # the producing normalize chain has always landed; each
                # attention-only stage start gets one filler group to cover
                # the exp pipeline-fill bubble.
                stage_work = {
                    0: {0: [L(K, 0, 1), L(V, 8)], 1: [L(V, 9), L(K, 0, 2)],
                        2: [L(V, 10), L(V, 11)], 3: [L(K, 0, 3), L(V, 12)],
                        4: [L(V, 13), L(K, 1, 0)], 5: [L(V, 14), L(V, 15)],
                        6: [L(K, 1, 1), L(K, 1, 2)],
                        7: [L(K, 1, 3), L(Q, 1, 0)]},
                    1: {0: [L(Q, 0, 1)], 1: [L(Q, 1, 1)], 2: [L(K, 2, 0)],
                        3: [L(K, 2, 1)], 4: [L(K, 2, 2)], 5: [L(K, 2, 3)],
                        6: [L(Q, 2, 0)]},
                    2: {0: [L(K, 3, 0)], 1: [L(K, 3, 1)], 2: [L(K, 3, 2)],
                        3: [L(K, 3, 3)], 4: [L(Q, 3, 0)]},
                    3: {0: [L(Q, 2, 1)]},
                    4: {0: [L(Q, 3, 1)], 5: [O(0)], 6: [O(1)]},
                    5: {0: [O(2)], 5: [O(3)]},
                    6: {0: [O(4)], 5: [O(5)]},
                    7: {0: [O(6)], 5: [O(7)]},
                }

                pending = None
                for idx in range(8):
                    q2, h = idx // 4, idx % 4
                    if masked and h == 0:
                        for j in range(LKC):
                            mt = maskp.tile([P, NQ], f32, name=f"m{j}")
                            nc.sync.dma_start(
                                mt[:],
                                maskT[j * P:(j + 1) * P,
                                      q2 * NQ:(q2 + 1) * NQ])
                            mask_tiles[j] = mt
                    work = stage_work.get(idx, {})
                    for jp in range(LKC // 2):
                        for w in work.get(jp, []):
                            w()
                        attn_pair(q2, h, jp)
                        if jp == 0 and pending is not None:
                            # finish the previous stage's normalize off the
                            # critical path
                            attn_norm_finish(*pending)
                            pending = None
                    pvraw, rsb = attn_norm_start(q2, h)
                    pending = (q2, h, pvraw, rsb)

                attn_norm_finish(*pending)
                # Tail: second-half output projection.
                for n in range(8, 16):
                    oproj_group(n // 2, n % 2, n)

    nc.compile()
    return nc


def _get_built(masked):
    if masked not in _BUILT:
        _BUILT[masked] = _build(masked)
    return _BUILT[masked]


def _shard_inputs(inputs, masked):
    import ml_dtypes

    bf16 = ml_dtypes.bfloat16

    x = np.asarray(inputs["mhca_input"], np.float32)
    enc = np.asarray(inputs["encoder_output"], np.float32)
    mask = np.asarray(inputs["cross_mask"], np.float32)
    W_kv = np.asarray(inputs["W_kv"], np.float32)
    b_kv = np.asarray(inputs["b_kv"], np.float32)
    W_q = np.asarray(inputs["W_q"], np.float32)
    b_q = np.asarray(inputs["b_q"], np.float32)
    W_o = np.asarray(inputs["W_o"], np.float32)

    scale = 1.0 / math.sqrt(DH)
    in_maps = []
    for c in range(N_CORES):
        b = c // 2
        g = c % 2
        heads = list(range(g * HPC, (g + 1) * HPC))
        sl = slice(g * OQ, (g + 1) * OQ)
        k_rows = np.concatenate(
            [W_kv[h * 2 * DH:h * 2 * DH + DH] for h in heads], 0)
        v_rows = np.concatenate(
            [W_kv[h * 2 * DH + DH:(h + 1) * 2 * DH] for h in heads], 0)
        bv_rows = np.concatenate(
            [b_kv[h * 2 * DH + DH:(h + 1) * 2 * DH] for h in heads], 0)
        xT = np.ascontiguousarray(x[b].T)      # [1024, 1024]
        encT = np.ascontiguousarray(enc[b].T)  # [1024, 2048]
        m = {
            "wk": _repack(k_rows.T).astype(bf16),
            "wv": _repack(v_rows.T).astype(bf16),
            "wq": _repack((W_q[sl] * scale).T).astype(bf16),
            "wo": _repack(W_o[:, sl].T).astype(bf16),
            "smf": np.concatenate(
                [np.stack([b_kv[h * 2 * DH:h * 2 * DH + DH] for h in heads],
                          1),
                 (b_q[sl] * scale).reshape(HPC, DH).T,
                 np.tile(bv_rows[None, :], (P, 1)),
                 np.eye(P)], axis=1).astype(np.float32),
            "ones": np.ones((P, 1), bf16),
        }
        for q in range(4):
            m[f"enc{q}"] = _repack(encT[:, q * NQ:(q + 1) * NQ]).astype(bf16)
        for q in range(2):
            m[f"x{q}"] = _repack(xT[:, q * NQ:(q + 1) * NQ]).astype(bf16)
        if masked:
            m["maskT"] = np.ascontiguousarray(mask[b].T)
        in_maps.append(m)
    return in_maps


def kernel(mhca_input, encoder_output, cross_mask, W_kv, b_kv, W_q, b_q, W_o,
           b_o):
    from concourse.bass_utils import run_bass_kernel_spmd

    inputs = {
        "mhca_input": mhca_input, "encoder_output": encoder_output,
        "cross_mask": cross_mask, "W_kv": W_kv, "b_kv": b_kv, "W_q": W_q,
        "b_q": b_q, "W_o": W_o,
    }
    b_o = np.asarray(b_o, np.float32)
    masked = bool(np.any(np.asarray(cross_mask)))
    nc = _get_built(masked)
    in_maps = _shard_inputs(inputs, masked)

    res = run_bass_kernel_spmd(nc, in_maps, core_ids=list(range(N_CORES)))
    outs = [np.asarray(res.results[c]["out"], np.float32)
            for c in range(N_CORES)]
    full = np.stack([outs[2 * b] + outs[2 * b + 1] for b in range(B)], 0)
    return (full + b_o[None, None, :]).astype(np.float32)


# revision 21
# speedup vs baseline: 1.4122x; 1.0773x over previous
"""Multi-head cross-attention on 8 Trainium2 NeuronCores.

Problem shapes (hardcoded): B=4, Ld=1024, Le=2048, d_model=1024, 8 heads x 128.
Sharding: core c handles batch b=c//2 and head-group g=c%2 (4 heads each).
Each core computes q/k/v projections for its heads, attention, and a partial
output projection over its heads' value dims; the host sums the two partial
outputs per batch and adds b_o.

All inputs are host-repacked into contiguous [128, X] layouts so each tensor
is a single large DMA (descriptor-generation slots are an exclusive 625ns
resource; dozens of small DMAs serialize the front of the kernel).

Schedule: one software-pipelined stream.  Projection bank-groups (K/Q/V and
later the output projection) are queued and drained a few per attention
chunk-pair, so the PE never waits for the Act engine's exp stream and the
whole kernel is PE-bound.  DMA order feeds the V path first (wk, enc, wv),
giving the PE continuous early work while wq/x stream in.

Softmax denominators use tall-skinny matmuls: pT is the *stationary* operand
and a ones column moves, so each [128,1] per-query partial sum costs ~1 PE
row instead of the 512 rows a [1,512] ones-stationary layout costs.

Exps are issued per chunk-pair over a [128,1024] two-bank PSUM span (halving
the Act engine's fixed access overhead), double-buffered across two spans so
scores never wait on the previous exp.

The per-stage normalize chain (reciprocal -> PE-transpose to a [1,512] row
-> gpsimd partition-broadcast -> multiply) is split: the PV accumulator is
drained to SBUF raw (one DVE copy) so the next stage's PV can start
immediately, and the rest of the chain is emitted after the next stage's
first pair, fully off the PE's critical path.

PSUM banks: 0-3 two score pair-spans, 4 PV accumulator, 5 denominator
columns, 6-7 projection / output-projection accumulators (rotating).
"""

import math
import sys

import numpy as np

for _p in ("/opt/trn_rl_repo", "/root/.axon_site/_ro/trn_rl_repo"):
    if _p not in sys.path:
        sys.path.append(_p)

B = 4
LQ = 1024
LK = 2048
D = 1024
H = 8
DH = 128
P = 128
HPC = 4          # heads per core
OQ = HPC * DH    # 512 projected dims per core
NQ = 512         # matmul moving free dim
KC = D // P      # 8 contraction chunks for projections
LKC = LK // P    # 16 key chunks
N_CORES = 8

_BUILT = {}


def _repack(a):
    """[KC*128, X] -> [128, KC*X] with d-chunk-major columns."""
    kc = a.shape[0] // P
    return np.ascontiguousarray(
        a.reshape(kc, P, a.shape[1]).transpose(1, 0, 2).reshape(P, -1))


def _build(masked):
    import concourse.bass as bass  # noqa: F401
    import concourse.tile as tile
    import concourse.mybir as mybir
    from concourse import bacc

    f32 = mybir.dt.float32
    bf16 = mybir.dt.bfloat16
    f32r = mybir.dt.float32r
    Exp = mybir.ActivationFunctionType.Exp
    Copy = mybir.ActivationFunctionType.Copy

    nc = bacc.Bacc("TRN2", target_bir_lowering=False, debug=False,
                   num_devices=N_CORES)

    BW = KC * NQ  # 4096: big packed width
    x_d = [nc.dram_tensor(f"x{q}", [P, BW], bf16, kind="ExternalInput").ap()
           for q in range(2)]
    e_d = [nc.dram_tensor(f"enc{q}", [P, BW], bf16, kind="ExternalInput").ap()
           for q in range(4)]
    wk_d = nc.dram_tensor("wk", [P, BW], bf16, kind="ExternalInput").ap()
    wv_d = nc.dram_tensor("wv", [P, BW], bf16, kind="ExternalInput").ap()
    wq_d = nc.dram_tensor("wq", [P, BW], bf16, kind="ExternalInput").ap()
    wo_d = nc.dram_tensor("wo", [P, BW], bf16, kind="ExternalInput").ap()
    # bk | bq | bvb | ident packed as one f32 DMA (cols 0:4, 4:8, 8:520,
    # 520:648); ones is bf16 and rides separately.
    smf_d = nc.dram_tensor("smf", [P, 648], f32, kind="ExternalInput").ap()
    ones_d = nc.dram_tensor("ones", [P, 1], bf16, kind="ExternalInput").ap()
    if masked:
        maskT = nc.dram_tensor("maskT", [LK, LQ], f32, kind="ExternalInput").ap()
    out_d = nc.dram_tensor("out", [LQ, D], bf16, kind="ExternalOutput").ap()

    with tile.TileContext(nc) as tc:
        with tc.tile_pool(name="persist", bufs=1) as persist:
            qT = [[persist.tile([P, NQ], f32r, name=f"qT{h}_{q}")
                   for q in range(2)] for h in range(HPC)]
            kT = [[persist.tile([P, NQ], f32r, name=f"kT{h}_{lk}")
                   for lk in range(4)] for h in range(HPC)]
            vch = [persist.tile([P, OQ], bf16, name=f"v{j}") for j in range(LKC)]
            smf = persist.tile([P, 648], f32, name="smf")
            bk_sb = smf[:, 0:HPC]
            bq_sb = smf[:, HPC:2 * HPC]
            bv_sb = smf[:, 2 * HPC:2 * HPC + OQ]
            ident = smf[:, 2 * HPC + OQ:2 * HPC + OQ + P]
            ones_col = persist.tile([P, 1], bf16, name="ones")
            warm = persist.tile([1, HPC], f32, name="warm")
            wkb = persist.tile([P, BW], bf16, name="wkb")
            wvb = persist.tile([P, BW], bf16, name="wvb")
            wqb = persist.tile([P, BW], bf16, name="wqb")
            wob = persist.tile([P, BW], bf16, name="wob")
            eb = [persist.tile([P, BW], bf16, name=f"eb{q}") for q in range(4)]
            xb = [persist.tile([P, BW], bf16, name=f"xb{q}") for q in range(2)]
            valsT = [[persist.tile([P, NQ], bf16, name=f"valsT{h}_{q}")
                      for q in range(2)] for h in range(HPC)]

            with (
                tc.tile_pool(name="acc", bufs=1, space="PSUM") as acc,
                tc.tile_pool(name="pTp", bufs=3) as pTp,
                tc.tile_pool(name="smallp", bufs=2) as smallp,
                tc.tile_pool(name="maskp", bufs=16 if masked else 1) as maskp,
                tc.tile_pool(name="osb", bufs=4) as osb,
            ):
                # PSUM: two score pair-spans (banks 0-3), single PV
                # accumulator (bank 4), denominator columns (bank 5),
                # projection/out-proj accumulators (banks 6-7).
                sp = [acc.tile([P, 2 * NQ], f32, name=f"sp{t}")
                      for t in range(2)]
                pvb = acc.tile([P, NQ], f32, name="pvb")
                dbk = acc.tile([P, NQ], f32, name="dbk")
                pj = [acc.tile([P, NQ], f32, name=f"pj{t}") for t in range(2)]

                # Warm-up: absorb the PE p-state ramp on 1-column matmuls
                # against a memset tile (no DMA dependency), and preload the
                # Act Exp table the same way.
                wt = persist.tile([P, 4], bf16, name="wt")
                nc.vector.memset(wt[:], 1.0)
                for _ in range(4):
                    nc.tensor.matmul(pj[1][:1, :1], wt[:, :1], wt[:, :1],
                                     start=True, stop=True)
                nc.scalar.activation(warm[:], wt[:1, :], Exp)
                # ---- DMA issue order == service order: V path first, and
                # the first K-group's tensors split in half for an earlier
                # start. Small tensors ride in the first gap they're needed.
                hw_ = BW // 2
                nc.sync.dma_start(wkb[:, :hw_], wk_d[:, :hw_])
                nc.sync.dma_start(eb[0][:, :hw_], e_d[0][:, :hw_])
                nc.sync.dma_start(wkb[:, hw_:], wk_d[:, hw_:])
                nc.sync.dma_start(eb[0][:, hw_:], e_d[0][:, hw_:])
                nc.sync.dma_start(smf[:], smf_d[:])
                nc.sync.dma_start(ones_col[:], ones_d[:])
                nc.sync.dma_start(wvb[:], wv_d[:])
                nc.sync.dma_start(eb[1][:], e_d[1][:])
                nc.sync.dma_start(wqb[:], wq_d[:])
                nc.sync.dma_start(xb[0][:], x_d[0][:])
                nc.sync.dma_start(eb[2][:], e_d[2][:])
                nc.sync.dma_start(eb[3][:], e_d[3][:])
                nc.sync.dma_start(xb[1][:], x_d[1][:])
                nc.sync.dma_start(wob[:], wo_d[:])

                # ---- projection bank-group emitters (banks 6-7 rotating)
                nbg = [0]

                def next_pj():
                    bank = pj[nbg[0] % 2]
                    nbg[0] += 1
                    return bank

                def kproj_group(h, lk):
                    bank = next_pj()
                    for d in range(KC):
                        nc.tensor.matmul(
                            bank[:],
                            wkb[:, d * OQ + h * DH:d * OQ + (h + 1) * DH],
                            eb[lk][:, d * NQ:(d + 1) * NQ],
                            start=(d == 0), stop=(d == KC - 1))
                    nc.vector.tensor_scalar_add(
                        kT[h][lk][:], bank[:], bk_sb[:, h:h + 1])

                def qproj_group(h, q2):
                    bank = next_pj()
                    for d in range(KC):
                        nc.tensor.matmul(
                            bank[:],
                            wqb[:, d * OQ + h * DH:d * OQ + (h + 1) * DH],
                            xb[q2][:, d * NQ:(d + 1) * NQ],
                            start=(d == 0), stop=(d == KC - 1))
                    nc.vector.tensor_scalar_add(
                        qT[h][q2][:], bank[:], bq_sb[:, h:h + 1])

                def vproj_group(j):
                    bank = next_pj()
                    for d in range(KC):
                        nc.tensor.matmul(
                            bank[:],
                            eb[j // 4][:, d * NQ + (j % 4) * P:
                                       d * NQ + (j % 4 + 1) * P],
                            wvb[:, d * OQ:(d + 1) * OQ],
                            start=(d == 0), stop=(d == KC - 1))
                    nc.vector.tensor_add(vch[j][:], bank[:], bv_sb[:])

                def oproj_group(lqc, o2, n):
                    bank = next_pj()
                    for h in range(HPC):
                        nc.tensor.matmul(
                            bank[:],
                            valsT[h][lqc // 4][:, (lqc % 4) * P:
                                                (lqc % 4 + 1) * P],
                            wob[:, h * D + o2 * NQ:h * D + (o2 + 1) * NQ],
                            start=(h == 0), stop=(h == HPC - 1))
                    ot = osb.tile([P, NQ], bf16, name="ot")
                    if n % 2 == 0:
                        nc.vector.tensor_copy(ot[:], bank[:])
                    else:
                        nc.scalar.activation(ot[:], bank[:], Copy)
                    nc.sync.dma_start(
                        out_d[lqc * P:(lqc + 1) * P,
                              o2 * NQ:(o2 + 1) * NQ], ot[:])

                # ---- attention emitters
                mask_tiles = [None] * LKC

                def attn_pair(q2, h, jp):
                    """Chunks j=2jp,2jp+1: scores into pair-span jp%2, one
                    exp over both, then PV + denominator matmuls."""
                    span = sp[jp % 2]
                    for t in range(2):
                        j = 2 * jp + t
                        nc.tensor.matmul(
                            span[:, t * NQ:(t + 1) * NQ],
                            kT[h][j // 4][:, (j % 4) * P:(j % 4 + 1) * P],
                            qT[h][q2][:],
                            start=True, stop=True)
                        if masked:
                            nc.vector.tensor_add(
                                span[:, t * NQ:(t + 1) * NQ],
                                span[:, t * NQ:(t + 1) * NQ],
                                mask_tiles[j][:])
                    pT2 = pTp.tile([P, 2 * NQ], bf16, name="pT2")
                    nc.scalar.activation(pT2[:], span[:], Exp)
                    for t in range(2):
                        j = 2 * jp + t
                        nc.tensor.matmul(
                            pvb[:],
                            vch[j][:, h * DH:(h + 1) * DH],
                            pT2[:, t * NQ:(t + 1) * NQ],
                            start=(j == 0), stop=(j == LKC - 1))
                        for s in range(4):
                            nc.tensor.matmul(
                                dbk[:, s:s + 1],
                                pT2[:, t * NQ + s * P:t * NQ + (s + 1) * P],
                                ones_col[:],
                                start=(j == 0 and s == 0),
                                stop=(j == LKC - 1 and s == 3),
                                skip_group_check=True)

                def attn_norm_start(q2, h):
                    """Free the PV/denominator banks: raw-copy the PV
                    accumulator and take the reciprocal of d."""
                    pvraw = smallp.tile([P, NQ], f32, name="pvraw")
                    nc.vector.tensor_copy(pvraw[:], pvb[:])
                    rsb = smallp.tile([P, 4], f32, name="rsb")
                    nc.vector.reciprocal(rsb[:], dbk[:, 0:4])
                    return pvraw, rsb

                def attn_norm_finish(q2, h, pvraw, rsb):
                    """1/d -> [1,512] row via PE transposes (into a rotating
                    projection bank) -> partition broadcast -> scale."""
                    tb = next_pj()
                    for s in range(4):
                        nc.tensor.transpose(
                            tb[0:1, s * P:(s + 1) * P],
                            rsb[:, s:s + 1], ident[:])
                    rrow = smallp.tile([1, NQ], f32, name="rrow")
                    nc.vector.tensor_copy(rrow[:], tb[0:1, :])
                    bcast = smallp.tile([P, NQ], f32, name="bcast")
                    nc.gpsimd.partition_broadcast(bcast[:], rrow[:])
                    nc.vector.tensor_mul(
                        valsT[h][q2][:], pvraw[:], bcast[:])

                # ---- the pipelined schedule.
                # Pre-stage: first K group, the V stream, then Q(h0,0).
                kproj_group(0, 0)
                for j in range(8):
                    vproj_group(j)
                qproj_group(0, 0)

                K, Q, V = kproj_group, qproj_group, vproj_group

                def O(n):
                    return lambda: oproj_group(n // 2, n % 2, n)

                def L(f, *a):
                    return lambda: f(*a)

                # Per-stage, per-pair-slot work placement.  Projections lead
                # the attention that consumes them; out-proj groups go at
                # jp>=5 of their earliest stage or jp=0 of later stages so
                # the producing normalize chain has always landed; each
                # attention-only stage start gets one filler group to cover
                # the exp pipeline-fill bubble.
                stage_work = {
                    0: {0: [L(K, 0, 1), L(V, 8)], 1: [L(V, 9), L(K, 0, 2)],
                        2: [L(V, 10), L(V, 11)], 3: [L(K, 0, 3), L(V, 12)],
                        4: [L(V, 13), L(K, 1, 0)], 5: [L(V, 14), L(V, 15)],
                        6: [L(K, 1, 1), L(K, 1, 2)],
                        7: [L(K, 1, 3), L(Q, 1, 0)]},
                    1: {0: [L(Q, 0, 1)], 1: [L(Q, 1, 1)], 2: [L(K, 2, 0)],
                        3: [L(K, 2, 1)], 4: [L(K, 2, 2)], 5: [L(K, 2, 3)],
                        6: [L(Q, 2, 0)]},
                    2: {0: [L(K, 3, 0)], 1: [L(K, 3, 1)], 2: [L(K, 3, 2)],
                        3: [L(K, 3, 3)], 4: [L(Q, 3, 0)]},
                    3: {0: [L(Q, 2, 1)]},
                    4: {0: [L(Q, 3, 1)], 5: [O(0)], 6: [O(1)]},
                    5: {0: [O(2)], 5: [O(3)]},
                    6: {0: [O(4)], 5: [O(5)]},
                    7: {0: [O(6)], 5: [O(7)]},
                }

                pending = None
                for idx in range(8):
                    q2, h = idx // 4, idx % 4
                    if masked and h == 0:
                        for j in range(LKC):
                            mt = maskp.tile([P, NQ], f32, name=f"m{j}")
                            nc.sync.dma_start(
                                mt[:],
                                maskT[j * P:(j + 1) * P,
                                      q2 * NQ:(q2 + 1) * NQ])
                            mask_tiles[j] = mt
                    work = stage_work.get(idx, {})
                    for jp in range(LKC // 2):
                        for w in work.get(jp, []):
                            w()
                        attn_pair(q2, h, jp)
                        if jp == 0 and pending is not None:
                            # finish the previous stage's normalize off the
                            # critical path
                            attn_norm_finish(*pending)
                            pending = None
                    pvraw, rsb = attn_norm_start(q2, h)
                    pending = (q2, h, pvraw, rsb)

                attn_norm_finish(*pending)
                # Tail: second-half output projection.
                for n in range(8, 16):
                    oproj_group(n // 2, n % 2, n)

    nc.compile()
    return nc


def _get_built(masked):
    if masked not in _BUILT:
        _BUILT[masked] = _build(masked)
    return _BUILT[masked]


def _shard_inputs(inputs, masked):
    import ml_dtypes

    bf16 = ml_dtypes.bfloat16

    x = np.asarray(inputs["mhca_input"], np.float32)
    enc = np.asarray(inputs["encoder_output"], np.float32)
    mask = np.asarray(inputs["cross_mask"], np.float32)
    W_kv = np.asarray(inputs["W_kv"], np.float32)
    b_kv = np.asarray(inputs["b_kv"], np.float32)
    W_q = np.asarray(inputs["W_q"], np.float32)
    b_q = np.asarray(inputs["b_q"], np.float32)
    W_o = np.asarray(inputs["W_o"], np.float32)

    scale = 1.0 / math.sqrt(DH)
    in_maps = []
    for c in range(N_CORES):
        b = c // 2
        g = c % 2
        heads = list(range(g * HPC, (g + 1) * HPC))
        sl = slice(g * OQ, (g + 1) * OQ)
        k_rows = np.concatenate(
            [W_kv[h * 2 * DH:h * 2 * DH + DH] for h in heads], 0)
        v_rows = np.concatenate(
            [W_kv[h * 2 * DH + DH:(h + 1) * 2 * DH] for h in heads], 0)
        bv_rows = np.concatenate(
            [b_kv[h * 2 * DH + DH:(h + 1) * 2 * DH] for h in heads], 0)
        xT = np.ascontiguousarray(x[b].T)      # [1024, 1024]
        encT = np.ascontiguousarray(enc[b].T)  # [1024, 2048]
        m = {
            "wk": _repack(k_rows.T).astype(bf16),
            "wv": _repack(v_rows.T).astype(bf16),
            "wq": _repack((W_q[sl] * scale).T).astype(bf16),
            "wo": _repack(W_o[:, sl].T).astype(bf16),
            "smf": np.concatenate(
                [np.stack([b_kv[h * 2 * DH:h * 2 * DH + DH] for h in heads],
                          1),
                 (b_q[sl] * scale).reshape(HPC, DH).T,
                 np.tile(bv_rows[None, :], (P, 1)),
                 np.eye(P)], axis=1).astype(np.float32),
            "ones": np.ones((P, 1), bf16),
        }
        for q in range(4):
            m[f"enc{q}"] = _repack(encT[:, q * NQ:(q + 1) * NQ]).astype(bf16)
        for q in range(2):
            m[f"x{q}"] = _repack(xT[:, q * NQ:(q + 1) * NQ]).astype(bf16)
        if masked:
            m["maskT"] = np.ascontiguousarray(mask[b].T)
        in_maps.append(m)
    return in_maps


def kernel(mhca_input, encoder_output, cross_mask, W_kv, b_kv, W_q, b_q, W_o,
           b_o):
    from concourse.bass_utils import run_bass_kernel_spmd

    inputs = {
        "mhca_input": mhca_input, "encoder_output": encoder_output,
        "cross_mask": cross_mask, "W_kv": W_kv, "b_kv": b_kv, "W_q": W_q,
        "b_q": b_q, "W_o": W_o,
    }
    b_o = np.asarray(b_o, np.float32)
    masked = bool(np.any(np.asarray(cross_mask)))
    nc = _get_built(masked)
    in_maps = _shard_inputs(inputs, masked)

    res = run_bass_kernel_spmd(nc, in_maps, core_ids=list(range(N_CORES)))
    outs = [np.asarray(res.results[c]["out"], np.float32)
            for c in range(N_CORES)]
    full = np.stack([outs[2 * b] + outs[2 * b + 1] for b in range(B)], 0)
    return (full + b_o[None, None, :]).astype(np.float32)


# revision 23
# speedup vs baseline: 1.5604x; 1.1050x over previous
"""Multi-head cross-attention on 8 Trainium2 NeuronCores.

Problem shapes (hardcoded): B=4, Ld=1024, Le=2048, d_model=1024, 8 heads x 128.
Sharding: core c handles batch b=c//2 and head-group g=c%2 (4 heads each).
Each core computes q/k/v projections for its heads, attention, and a partial
output projection over its heads' value dims; the host sums the two partial
outputs per batch and adds b_o.

Projections run as split-fp8 DoubleRow matmuls: every operand is host-split
into hi+lo fp8e4 parts (hi = fp8(s*x), lo = fp8(s*x - hi), one power-of-2
scale s per tensor chosen for e4m3's normal range).  A DoubleRow matmul
contracts two 128-partition chunks per pass at 0.5 cycles/row, and the three
products hi*hi, hi*lo, lo*hi (lo*lo is ~2^-8 relative, dropped) cover a
d-chunk pair in 3 matmuls = 0.75x the bf16 cost at bf16-class accuracy.
PSUM drains rescale by the product of the operand scales (fused into the
bias-add on the DVE).  The output projection does the same with vals split
on-chip by the normalize chain.

All inputs are host-repacked into contiguous [128, ...] layouts so each
tensor is one DMA (descriptor-generation slots are an exclusive ~625ns
resource).  DMA order feeds the V path first (wk, enc, wv) with hi parts
before lo parts, giving the PE continuous early work while wq/x stream in.

Schedule: one software-pipelined stream; projection bank-groups (K/Q/V and
later the output projection) are queued a few per attention chunk-pair so
the PE never waits for the Act engine's exp stream.

Softmax denominators use tall-skinny matmuls: pT is the *stationary* operand
and a ones column moves, so each [128,1] per-query partial sum costs ~1 PE
row instead of the 512 a [1,512] ones-stationary layout costs.

Exps are issued per chunk-pair over a [128,1024] two-bank PSUM span (halving
the Act engine's fixed access overhead), double-buffered across two spans so
scores never wait on the previous exp.

The per-stage normalize chain (reciprocal -> PE-transpose to a [1,512] row
in a rotating projection bank -> gpsimd partition-broadcast -> scaled
multiply + fp8 hi/lo split of vals) is split: the PV accumulator is drained
to SBUF raw (one DVE copy) so the next stage's PV can start immediately, and
the rest is emitted after the next stage's first pair, off the PE's
critical path.

PSUM banks: 0-3 two score pair-spans, 4 PV accumulator, 5 denominator
columns, 6-7 projection / output-projection accumulators (rotating).
"""

import math
import sys

import numpy as np

for _p in ("/opt/trn_rl_repo", "/root/.axon_site/_ro/trn_rl_repo"):
    if _p not in sys.path:
        sys.path.append(_p)

B = 4
LQ = 1024
LK = 2048
D = 1024
H = 8
DH = 128
P = 128
HPC = 4          # heads per core
OQ = HPC * DH    # 512 projected dims per core
NQ = 512         # matmul moving free dim
KC = D // P      # 8 contraction chunks for projections
LKC = LK // P    # 16 key chunks
N_CORES = 8

# fp8e4 (e4m3) per-tensor scales: picked so values sit in the normal range.
SA = 8.0       # x, enc ~ N(0,1)
SW = 128.0     # wk, wv (sigma 1/32)
SQ = 1024.0    # wq with 1/sqrt(dh) folded (sigma 1/256)
SO = 128.0     # wo (sigma 1/32)
SVAL = 32.0    # vals (sigma ~0.03), applied on-chip
KINV = 1.0 / (SA * SW)
VINV = 1.0 / (SA * SW)
QINV = 1.0 / (SA * SQ)
OINV = 1.0 / (SVAL * SO)

_BUILT = {}


def _repack(a):
    """[KC*128, X] -> [128, KC*X] with d-chunk-major columns."""
    kc = a.shape[0] // P
    return np.ascontiguousarray(
        a.reshape(kc, P, a.shape[1]).transpose(1, 0, 2).reshape(P, -1))


def _split8(a, s):
    import ml_dtypes
    f8 = ml_dtypes.float8_e4m3
    sa = (a * s).astype(np.float32)
    hi = sa.astype(f8)
    lo = (sa - hi.astype(np.float32)).astype(f8)
    return hi, lo


def _build(masked):
    import concourse.bass as bass  # noqa: F401
    import concourse.tile as tile
    import concourse.mybir as mybir
    from concourse import bacc

    f32 = mybir.dt.float32
    bf16 = mybir.dt.bfloat16
    f32r = mybir.dt.float32r
    fp8 = mybir.dt.float8e4
    DR = mybir.MatmulPerfMode.DoubleRow
    Exp = mybir.ActivationFunctionType.Exp
    MUL = mybir.AluOpType.mult
    ADD = mybir.AluOpType.add
    SUB = mybir.AluOpType.subtract

    nc = bacc.Bacc("TRN2", target_bir_lowering=False, debug=False,
                   num_devices=N_CORES)

    def din(name, shape, dt=fp8):
        return nc.dram_tensor(name, shape, dt, kind="ExternalInput").ap()

    x_d = [[din(f"x{q}{p}", [P, KC, NQ]) for p in range(2)] for q in range(2)]
    e_d = [[din(f"enc{q}{p}", [P, KC, NQ]) for p in range(2)]
           for q in range(4)]
    wk_d = [din(f"wk{p}", [P, KC, OQ]) for p in range(2)]
    wv_d = [din(f"wv{p}", [P, KC, OQ]) for p in range(2)]
    wq_d = [din(f"wq{p}", [P, KC, OQ]) for p in range(2)]
    wo_d = [din(f"wo{p}", [P, HPC, D]) for p in range(2)]
    # bk | bq | bvb | ident packed as one f32 DMA.
    smf_d = din("smf", [P, 648], f32)
    ones_d = din("ones", [P, 1], bf16)
    if masked:
        maskT = din("maskT", [LK, LQ], f32)
    out_d = nc.dram_tensor("out", [LQ, D], bf16, kind="ExternalOutput").ap()

    with tile.TileContext(nc) as tc:
        with tc.tile_pool(name="persist", bufs=1) as persist:
            qT = [[persist.tile([P, NQ], f32r, name=f"qT{h}_{q}")
                   for q in range(2)] for h in range(HPC)]
            kT = [[persist.tile([P, NQ], f32r, name=f"kT{h}_{lk}")
                   for lk in range(4)] for h in range(HPC)]
            vch = [persist.tile([P, OQ], bf16, name=f"v{j}") for j in range(LKC)]
            smf = persist.tile([P, 648], f32, name="smf")
            bk_sb = smf[:, 0:HPC]
            bq_sb = smf[:, HPC:2 * HPC]
            bv_sb = smf[:, 2 * HPC:2 * HPC + OQ]
            ident = smf[:, 2 * HPC + OQ:2 * HPC + OQ + P]
            ones_col = persist.tile([P, 1], bf16, name="ones")
            warm = persist.tile([1, HPC], f32, name="warm")
            wkb = [persist.tile([P, KC, OQ], fp8, name=f"wkb{p}")
                   for p in range(2)]
            wvb = [persist.tile([P, KC, OQ], fp8, name=f"wvb{p}")
                   for p in range(2)]
            wqb = [persist.tile([P, KC, OQ], fp8, name=f"wqb{p}")
                   for p in range(2)]
            wob = [persist.tile([P, HPC, D], fp8, name=f"wob{p}")
                   for p in range(2)]
            eb = [[persist.tile([P, KC, NQ], fp8, name=f"eb{q}_{p}")
                   for p in range(2)] for q in range(4)]
            xb = [[persist.tile([P, KC, NQ], fp8, name=f"xb{q}_{p}")
                   for p in range(2)] for q in range(2)]
            vals = [[persist.tile([P, HPC, NQ], fp8, name=f"vals{q}_{p}")
                     for p in range(2)] for q in range(2)]

            with (
                tc.tile_pool(name="acc", bufs=1, space="PSUM") as acc,
                tc.tile_pool(name="pTp", bufs=3) as pTp,
                tc.tile_pool(name="smallp", bufs=2) as smallp,
                tc.tile_pool(name="maskp", bufs=16 if masked else 1) as maskp,
                tc.tile_pool(name="osb", bufs=4) as osb,
            ):
                # PSUM: two score pair-spans (banks 0-3), single PV
                # accumulator (bank 4), denominator columns (bank 5),
                # projection/out-proj accumulators (banks 6-7).
                sp = [acc.tile([P, 2 * NQ], f32, name=f"sp{t}")
                      for t in range(2)]
                pvb = acc.tile([P, NQ], f32, name="pvb")
                dbk = acc.tile([P, NQ], f32, name="dbk")
                pj = [acc.tile([P, NQ], f32, name=f"pj{t}") for t in range(2)]

                # Warm-up: absorb the PE p-state ramp on 1-column matmuls
                # against a memset tile (no DMA dependency), and preload the
                # Act Exp table the same way.
                wt = persist.tile([P, 4], bf16, name="wt")
                nc.vector.memset(wt[:], 1.0)
                for _ in range(4):
                    nc.tensor.matmul(pj[1][:1, :1], wt[:, :1], wt[:, :1],
                                     start=True, stop=True)
                nc.scalar.activation(warm[:], wt[:1, :], Exp)
                # ---- DMA issue order == service order: V path first,
                # hi parts before lo parts.
                nc.sync.dma_start(wkb[0][:], wk_d[0][:])
                nc.sync.dma_start(eb[0][0][:], e_d[0][0][:])
                nc.sync.dma_start(wkb[1][:], wk_d[1][:])
                nc.sync.dma_start(eb[0][1][:], e_d[0][1][:])
                nc.sync.dma_start(smf[:], smf_d[:])
                nc.sync.dma_start(ones_col[:], ones_d[:])
                for p in range(2):
                    nc.sync.dma_start(wvb[p][:], wv_d[p][:])
                for p in range(2):
                    nc.sync.dma_start(eb[1][p][:], e_d[1][p][:])
                for p in range(2):
                    nc.sync.dma_start(wqb[p][:], wq_d[p][:])
                for p in range(2):
                    nc.sync.dma_start(xb[0][p][:], x_d[0][p][:])
                for p in range(2):
                    nc.sync.dma_start(eb[2][p][:], e_d[2][p][:])
                for p in range(2):
                    nc.sync.dma_start(eb[3][p][:], e_d[3][p][:])
                for p in range(2):
                    nc.sync.dma_start(xb[1][p][:], x_d[1][p][:])
                for p in range(2):
                    nc.sync.dma_start(wob[p][:], wo_d[p][:])

                # ---- projection bank-group emitters (banks 6-7 rotating).
                # Each contraction d-chunk pair takes 3 DoubleRow matmuls:
                # hi*hi, lo(w)*hi, hi*lo (lo*lo dropped).  The hi*hi pass for
                # all pairs is emitted first so it can start as soon as the
                # hi DMAs land.
                nbg = [0]

                def next_pj():
                    bank = pj[nbg[0] % 2]
                    nbg[0] += 1
                    return bank

                def dr_group(bank, wts, mov, wslice, mslice):
                    """wts/mov: [hi, lo] tile lists; slices by d-pair dp."""
                    np_ = KC // 2
                    for wp, mp, first, last in ((0, 0, True, False),
                                                (1, 0, False, False),
                                                (0, 1, False, True)):
                        for dp in range(np_):
                            nc.tensor.matmul(
                                bank[:],
                                wts[wp][wslice(dp)],
                                mov[mp][mslice(dp)],
                                start=(first and dp == 0),
                                stop=(last and dp == np_ - 1),
                                perf_mode=DR)

                def kproj_group(h, lk):
                    bank = next_pj()
                    dr_group(
                        bank, wkb, eb[lk],
                        lambda dp: np.s_[:, 2 * dp:2 * dp + 2,
                                         h * DH:(h + 1) * DH],
                        lambda dp: np.s_[:, 2 * dp:2 * dp + 2, :])
                    nc.vector.tensor_scalar(
                        kT[h][lk][:], bank[:], KINV, bk_sb[:, h:h + 1],
                        MUL, ADD)

                def qproj_group(h, q2):
                    bank = next_pj()
                    dr_group(
                        bank, wqb, xb[q2],
                        lambda dp: np.s_[:, 2 * dp:2 * dp + 2,
                                         h * DH:(h + 1) * DH],
                        lambda dp: np.s_[:, 2 * dp:2 * dp + 2, :])
                    nc.vector.tensor_scalar(
                        qT[h][q2][:], bank[:], QINV, bq_sb[:, h:h + 1],
                        MUL, ADD)

                def vproj_group(j):
                    bank = next_pj()
                    dr_group(
                        bank, eb[j // 4], wvb,
                        lambda dp: np.s_[:, 2 * dp:2 * dp + 2,
                                         (j % 4) * P:(j % 4 + 1) * P],
                        lambda dp: np.s_[:, 2 * dp:2 * dp + 2, :])
                    nc.vector.scalar_tensor_tensor(
                        vch[j][:], bank[:], VINV, bv_sb[:], MUL, ADD)

                def oproj_group(lqc, o2, n):
                    bank = next_pj()
                    q2g = lqc // 4
                    for wp, mp, first, last in ((0, 0, True, False),
                                                (1, 0, False, False),
                                                (0, 1, False, True)):
                        for hp in range(HPC // 2):
                            nc.tensor.matmul(
                                bank[:],
                                vals[q2g][wp][:, 2 * hp:2 * hp + 2,
                                              (lqc % 4) * P:
                                              (lqc % 4 + 1) * P],
                                wob[mp][:, 2 * hp:2 * hp + 2,
                                        o2 * NQ:(o2 + 1) * NQ],
                                start=(first and hp == 0),
                                stop=(last and hp == HPC // 2 - 1),
                                perf_mode=DR)
                    ot = osb.tile([P, NQ], bf16, name="ot")
                    if n % 2 == 0:
                        nc.vector.tensor_scalar_mul(ot[:], bank[:], OINV)
                    else:
                        nc.scalar.mul(ot[:], bank[:], OINV)
                    nc.sync.dma_start(
                        out_d[lqc * P:(lqc + 1) * P,
                              o2 * NQ:(o2 + 1) * NQ], ot[:])

                # ---- attention emitters
                mask_tiles = [None] * LKC

                def attn_pair(q2, h, jp):
                    """Chunks j=2jp,2jp+1: scores into pair-span jp%2, one
                    exp over both, then PV + denominator matmuls."""
                    span = sp[jp % 2]
                    for t in range(2):
                        j = 2 * jp + t
                        nc.tensor.matmul(
                            span[:, t * NQ:(t + 1) * NQ],
                            kT[h][j // 4][:, (j % 4) * P:(j % 4 + 1) * P],
                            qT[h][q2][:],
                            start=True, stop=True)
                        if masked:
                            nc.vector.tensor_add(
                                span[:, t * NQ:(t + 1) * NQ],
                                span[:, t * NQ:(t + 1) * NQ],
                                mask_tiles[j][:])
                    pT2 = pTp.tile([P, 2 * NQ], bf16, name="pT2")
                    nc.scalar.activation(pT2[:], span[:], Exp)
                    for t in range(2):
                        j = 2 * jp + t
                        nc.tensor.matmul(
                            pvb[:],
                            vch[j][:, h * DH:(h + 1) * DH],
                            pT2[:, t * NQ:(t + 1) * NQ],
                            start=(j == 0), stop=(j == LKC - 1))
                        for s in range(4):
                            nc.tensor.matmul(
                                dbk[:, s:s + 1],
                                pT2[:, t * NQ + s * P:t * NQ + (s + 1) * P],
                                ones_col[:],
                                start=(j == 0 and s == 0),
                                stop=(j == LKC - 1 and s == 3),
                                skip_group_check=True)

                def attn_norm_start(q2, h):
                    """Free the PV/denominator banks: raw-copy the PV
                    accumulator and take the reciprocal of d."""
                    pvraw = smallp.tile([P, NQ], f32, name="pvraw")
                    nc.vector.tensor_copy(pvraw[:], pvb[:])
                    rsb = smallp.tile([P, 4], f32, name="rsb")
                    nc.vector.reciprocal(rsb[:], dbk[:, 0:4])
                    return pvraw, rsb

                def attn_norm_finish(q2, h, pvraw, rsb):
                    """1/d -> [1,512] row via PE transposes (into a rotating
                    projection bank) -> partition broadcast -> scaled
                    normalize, split into hi+lo fp8 for the out-proj."""
                    tb = next_pj()
                    for s in range(4):
                        nc.tensor.transpose(
                            tb[0:1, s * P:(s + 1) * P],
                            rsb[:, s:s + 1], ident[:])
                    rrow = smallp.tile([1, NQ], f32, name="rrow")
                    nc.vector.tensor_copy(rrow[:], tb[0:1, :])
                    bcast = smallp.tile([P, NQ], f32, name="bcast")
                    nc.gpsimd.partition_broadcast(bcast[:], rrow[:])
                    t2 = smallp.tile([P, NQ], f32, name="t2")
                    nc.vector.scalar_tensor_tensor(
                        t2[:], pvraw[:], SVAL, bcast[:], MUL, MUL)
                    vh = vals[q2][0][:, h:h + 1, :]
                    nc.vector.tensor_copy(vh, t2[:])
                    nc.vector.scalar_tensor_tensor(
                        vals[q2][1][:, h:h + 1, :], t2[:], 1.0, vh, MUL, SUB)

                # ---- the pipelined schedule.
                # Pre-stage: first K group, the V stream, then Q(h0,0).
                kproj_group(0, 0)
                for j in range(8):
                    vproj_group(j)
                qproj_group(0, 0)

                K, Q, V = kproj_group, qproj_group, vproj_group

                def O(n):
                    return lambda: oproj_group(n // 2, n % 2, n)

                def L(f, *a):
                    return lambda: f(*a)

                # Per-stage, per-pair-slot work placement.  Projections lead
                # the attention that consumes them; out-proj groups go at
                # jp>=5 of their earliest stage or jp=0 of later stages so
                # the producing normalize chain has always landed; each
                # attention-only stage start gets one filler group to cover
                # the exp pipeline-fill bubble.
                stage_work = {
                    0: {0: [L(K, 0, 1), L(V, 8)], 1: [L(V, 9), L(K, 0, 2)],
                        2: [L(V, 10), L(V, 11)], 3: [L(K, 0, 3), L(V, 12)],
                        4: [L(V, 13), L(K, 1, 0)], 5: [L(V, 14), L(V, 15)],
                        6: [L(K, 1, 1), L(K, 1, 2)],
                        7: [L(K, 1, 3), L(Q, 1, 0)]},
                    1: {0: [L(Q, 0, 1)], 1: [L(Q, 1, 1)], 2: [L(K, 2, 0)],
                        3: [L(K, 2, 1)], 4: [L(K, 2, 2)], 5: [L(K, 2, 3)],
                        6: [L(Q, 2, 0)]},
                    2: {0: [L(K, 3, 0)], 1: [L(K, 3, 1)], 2: [L(K, 3, 2)],
                        3: [L(K, 3, 3)], 4: [L(Q, 3, 0)]},
                    3: {0: [L(Q, 2, 1)]},
                    4: {0: [L(Q, 3, 1)], 5: [O(0)], 6: [O(1)]},
                    5: {0: [O(2)], 5: [O(3)]},
                    6: {0: [O(4)], 5: [O(5)]},
                    7: {0: [O(6)], 5: [O(7)]},
                }

                pending = None
                for idx in range(8):
                    q2, h = idx // 4, idx % 4
                    if masked and h == 0:
                        for j in range(LKC):
                            mt = maskp.tile([P, NQ], f32, name=f"m{j}")
                            nc.sync.dma_start(
                                mt[:],
                                maskT[j * P:(j + 1) * P,
                                      q2 * NQ:(q2 + 1) * NQ])
                            mask_tiles[j] = mt
                    work = stage_work.get(idx, {})
                    for jp in range(LKC // 2):
                        for w in work.get(jp, []):
                            w()
                        attn_pair(q2, h, jp)
                        if jp == 0 and pending is not None:
                            # finish the previous stage's normalize off the
                            # critical path
                            attn_norm_finish(*pending)
                            pending = None
                    pvraw, rsb = attn_norm_start(q2, h)
                    pending = (q2, h, pvraw, rsb)

                attn_norm_finish(*pending)
                # Tail: second-half output projection.
                for n in range(8, 16):
                    oproj_group(n // 2, n % 2, n)

    nc.compile()
    return nc


def _get_built(masked):
    if masked not in _BUILT:
        _BUILT[masked] = _build(masked)
    return _BUILT[masked]


def _shard_inputs(inputs, masked):
    import ml_dtypes

    bf16 = ml_dtypes.bfloat16

    x = np.asarray(inputs["mhca_input"], np.float32)
    enc = np.asarray(inputs["encoder_output"], np.float32)
    mask = np.asarray(inputs["cross_mask"], np.float32)
    W_kv = np.asarray(inputs["W_kv"], np.float32)
    b_kv = np.asarray(inputs["b_kv"], np.float32)
    W_q = np.asarray(inputs["W_q"], np.float32)
    b_q = np.asarray(inputs["b_q"], np.float32)
    W_o = np.asarray(inputs["W_o"], np.float32)

    scale = 1.0 / math.sqrt(DH)
    in_maps = []
    for c in range(N_CORES):
        b = c // 2
        g = c % 2
        heads = list(range(g * HPC, (g + 1) * HPC))
        sl = slice(g * OQ, (g + 1) * OQ)
        k_rows = np.concatenate(
            [W_kv[h * 2 * DH:h * 2 * DH + DH] for h in heads], 0)
        v_rows = np.concatenate(
            [W_kv[h * 2 * DH + DH:(h + 1) * 2 * DH] for h in heads], 0)
        bv_rows = np.concatenate(
            [b_kv[h * 2 * DH + DH:(h + 1) * 2 * DH] for h in heads], 0)
        xT = np.ascontiguousarray(x[b].T)      # [1024, 1024]
        encT = np.ascontiguousarray(enc[b].T)  # [1024, 2048]
        m = {
            "smf": np.concatenate(
                [np.stack([b_kv[h * 2 * DH:h * 2 * DH + DH] for h in heads],
                          1),
                 (b_q[sl] * scale).reshape(HPC, DH).T,
                 np.tile(bv_rows[None, :], (P, 1)),
                 np.eye(P)], axis=1).astype(np.float32),
            "ones": np.ones((P, 1), bf16),
        }
        for name, base, s in (("wk", _repack(k_rows.T), SW),
                              ("wv", _repack(v_rows.T), SW),
                              ("wq", _repack((W_q[sl] * scale).T), SQ),
                              ("wo", _repack(W_o[:, sl].T), SO)):
            hi, lo = _split8(base, s)
            m[name + "0"], m[name + "1"] = hi, lo
        for q in range(4):
            hi, lo = _split8(_repack(encT[:, q * NQ:(q + 1) * NQ]), SA)
            m[f"enc{q}0"], m[f"enc{q}1"] = hi, lo
        for q in range(2):
            hi, lo = _split8(_repack(xT[:, q * NQ:(q + 1) * NQ]), SA)
            m[f"x{q}0"], m[f"x{q}1"] = hi, lo
        if masked:
            m["maskT"] = np.ascontiguousarray(mask[b].T)
        in_maps.append(m)
    return in_maps


def kernel(mhca_input, encoder_output, cross_mask, W_kv, b_kv, W_q, b_q, W_o,
           b_o):
    from concourse.bass_utils import run_bass_kernel_spmd

    inputs = {
        "mhca_input": mhca_input, "encoder_output": encoder_output,
        "cross_mask": cross_mask, "W_kv": W_kv, "b_kv": b_kv, "W_q": W_q,
        "b_q": b_q, "W_o": W_o,
    }
    b_o = np.asarray(b_o, np.float32)
    masked = bool(np.any(np.asarray(cross_mask)))
    nc = _get_built(masked)
    in_maps = _shard_inputs(inputs, masked)

    res = run_bass_kernel_spmd(nc, in_maps, core_ids=list(range(N_CORES)))
    outs = [np.asarray(res.results[c]["out"], np.float32)
            for c in range(N_CORES)]
    full = np.stack([outs[2 * b] + outs[2 * b + 1] for b in range(B)], 0)
    return (full + b_o[None, None, :]).astype(np.float32)
